# revision 3
# baseline (speedup 1.0000x reference)
"""AttentionDCA pseudo-likelihood loss on 8 Trainium2 NeuronCores — fast path.

Mathematical structure exploited: with this problem's data distribution the
RBF kernel Vaa[h] = exp(-gamma*D2) is numerically an identity matrix
(off-diagonal entries < 1e-20, since D2 ~ 4096 and gamma = 1/21; diagonal
within 5e-5 of 1).  Writing Vaa[h] = I + R_h, the coupling tensor collapses:

    J[r,j,q,a] = Atot[r,j] * delta(q,a) + (residual, bounded by max R)
    Atot       = sum_h A[h],  diagonal (r==j) zeroed

so the energy tensor is 21 small matmuls instead of one 5376x5376 GEMM:

    E[q,r,m] = sum_j Atot[r,j] * [Z[j,m] == q]

and because sum_q [Z[j,m]==q] == 1, the planes satisfy
sum_q E[q,r,m] = rowsum(Atot)[r]: the device only computes 20 of the 21
planes and the host reconstructs the last one by subtraction.

Per core (M sharded 1024-per-core): 20 x (256x256)@(256x1024) fp16 matmuls
(~2.7 GFLOP, ~37 us of PE) with the one-hot RHS built on-device from Z by
DVE is_equal compares (4x 16-bit mode).  PSUM is drained fp32->fp16 split
5:3 across ACT:DVE; E returns in fp16 (10.5 MB); host does the cheap
logsumexp / gather / weighted-sum epilogue and the exact per-q L2 reg.

Tiered fallbacks, guarded by exact bounds on the dropped terms:
  1. shared-Atot 20-plane path   (needs off-diag Vaa ~ 0 AND uniform diag)
  2. per-q Atq 21-plane path     (needs off-diag Vaa ~ 0 only)
  3. dense J-matmul device path  (always valid; numpy as a last resort)
"""

import sys
import numpy as np

for p in ("/opt/trn_rl_repo", "/root/.axon_site/_ro/trn_rl_repo"):
    if p not in sys.path:
        sys.path.insert(0, p)

import ml_dtypes

import concourse.bass as bass
from concourse import bacc, mybir, tile
from concourse.bass_utils import run_bass_kernel_spmd

Q_AA = 21
H = 32
L = 256
DK = 32
M_TOT = 8192
N_CORES = 8
M_LOC = M_TOT // N_CORES          # 1024
F = L * Q_AA                      # 5376 flattened (pos, aa) dim
NB = F // 128                     # 42 blocks of 128
LAMBDA = 1e-3
N_PLANES = Q_AA - 1               # device computes 20, host rebuilds the 21st

_CACHE = {}


# --------------------------------------------------------------------------
# fast graph (shared Atot): E[q,r,m] = sum_j Atot[r,j] * [Z[j,m]==q],
# planes q = 0..19 only; plane 20 = rowsum(Atot) - sum(planes) on host
# --------------------------------------------------------------------------
def _build_fast_graph():
    if "fast" in _CACHE:
        return _CACHE["fast"]
    nc = bacc.Bacc(None, target_bir_lowering=False)
    # lhsT: at[k(128), kb, r] fp16 = Atot.T[kb*128+k, r]  (one 128 KB tile
    # shared by every plane)
    at_ext = nc.declare_dram_parameter(
        "at", [128, 2, L], mybir.dt.float16, isOutput=False
    )
    # z[k(128), kb, m] fp16 = Z[kb*128+k, m] as float
    z_ext = nc.declare_dram_parameter(
        "z", [128, 2, M_LOC], mybir.dt.float16, isOutput=False
    )
    # DRAM layout matches the SBUF tile's (partition-major) element order:
    # e[q, k, rb, m] with output row r = rb*128 + k
    e_ext = nc.declare_dram_parameter(
        "e", [N_PLANES, 128, 2, M_LOC], mybir.dt.float16, isOutput=True
    )

    with tile.TileContext(nc) as tc:
        with (
            tc.tile_pool(name="const", bufs=1) as cpool,
            tc.tile_pool(name="out", bufs=6) as opool,
            tc.tile_pool(name="psum", bufs=8, space=bass.MemorySpace.PSUM) as ppool,
        ):
            zt = cpool.tile([128, 2, M_LOC], mybir.dt.float16)
            at = cpool.tile([128, 2, L], mybir.dt.float16)
            # all one-hot planes stay resident; compares run a few planes
            # ahead of the matmul stream on DVE so the PE never waits
            # mid-flight (the cost model's p-state ramp rewards continuous
            # PE busy)
            zq = cpool.tile([128, N_PLANES, 2, M_LOC], mybir.dt.float16)

            nc.sync.dma_start(out=zt[:], in_=z_ext[:])
            nc.sync.dma_start(out=at[:], in_=at_ext[:])

            def _cmp(q):
                nc.vector.tensor_scalar(
                    zq[:, q], zt[:], float(q), None, mybir.AluOpType.is_equal
                )

            lookahead = 4
            for q in range(min(lookahead, N_PLANES)):
                _cmp(q)

            ci = 0
            for q in range(N_PLANES):
                if q + lookahead < N_PLANES:
                    _cmp(q + lookahead)
                otile = opool.tile([128, 2, M_LOC], mybir.dt.float16)
                for rb in range(2):
                    for mt in range(M_LOC // 512):
                        acc = ppool.tile([128, 512], mybir.dt.float32)
                        for kb in range(2):
                            nc.tensor.matmul(
                                acc[:],
                                at[:, kb, rb * 128:(rb + 1) * 128],
                                zq[:, q, kb, mt * 512:(mt + 1) * 512],
                                start=(kb == 0),
                                stop=(kb == 1),
                            )
                        # split the PSUM->SBUF drain 5:3 ACT:DVE so neither
                        # engine rate-limits the PE stream (DVE also runs
                        # the compares; gpsimd cannot read PSUM here);
                        # phase (1,4,6) chosen by exhaustive 3-of-8 sim sweep
                        if ci % 8 in (1, 4, 6):
                            nc.vector.tensor_copy(
                                otile[:, rb, mt * 512:(mt + 1) * 512], acc[:]
                            )
                        else:
                            nc.scalar.copy(
                                otile[:, rb, mt * 512:(mt + 1) * 512], acc[:]
                            )
                        ci += 1
                    # per-rb output DMAs pipeline the writeback finer and
                    # shrink the final drain after the last matmul
                    nc.sync.dma_start(out=e_ext[q, :, rb], in_=otile[:, rb])
    nc.finalize()   # Bacc.compile(): sync legalization + reg allocation
    _CACHE["fast"] = nc
    return nc


# --------------------------------------------------------------------------
# per-q graph: E[q,r,m] = sum_j Atq[q][r,j] * [Z[j,m]==q], all 21 planes
# (used when the Vaa diagonal is not uniform enough for the shared-Atot
# shortcut)
# --------------------------------------------------------------------------
def _build_perq_graph():
    if "perq" in _CACHE:
        return _CACHE["perq"]
    nc = bacc.Bacc(None, target_bir_lowering=False)
    at_ext = nc.declare_dram_parameter(
        "at", [Q_AA, 128, 2, L], mybir.dt.float16, isOutput=False
    )
    z_ext = nc.declare_dram_parameter(
        "z", [128, 2, M_LOC], mybir.dt.float16, isOutput=False
    )
    e_ext = nc.declare_dram_parameter(
        "e", [Q_AA, 128, 2, M_LOC], mybir.dt.float16, isOutput=True
    )

    with tile.TileContext(nc) as tc:
        with (
            tc.tile_pool(name="const", bufs=1) as cpool,
            tc.tile_pool(name="out", bufs=6) as opool,
            tc.tile_pool(name="psum", bufs=8, space=bass.MemorySpace.PSUM) as ppool,
        ):
            zt = cpool.tile([128, 2, M_LOC], mybir.dt.float16)
            at = cpool.tile([128, Q_AA, 2, L], mybir.dt.float16)
            zq = cpool.tile([128, Q_AA, 2, M_LOC], mybir.dt.float16)

            nc.sync.dma_start(out=zt[:], in_=z_ext[:])
            for q in range(Q_AA):
                nc.sync.dma_start(out=at[:, q], in_=at_ext[q])

            def _cmp(q):
                nc.vector.tensor_scalar(
                    zq[:, q], zt[:], float(q), None, mybir.AluOpType.is_equal
                )

            lookahead = 4
            for q in range(min(lookahead, Q_AA)):
                _cmp(q)

            ci = 0
            for q in range(Q_AA):
                if q + lookahead < Q_AA:
                    _cmp(q + lookahead)
                otile = opool.tile([128, 2, M_LOC], mybir.dt.float16)
                for rb in range(2):
                    for mt in range(M_LOC // 512):
                        acc = ppool.tile([128, 512], mybir.dt.float32)
                        for kb in range(2):
                            nc.tensor.matmul(
                                acc[:],
                                at[:, q, kb, rb * 128:(rb + 1) * 128],
                                zq[:, q, kb, mt * 512:(mt + 1) * 512],
                                start=(kb == 0),
                                stop=(kb == 1),
                            )
                        if ci % 3 == 2:
                            nc.vector.tensor_copy(
                                otile[:, rb, mt * 512:(mt + 1) * 512], acc[:]
                            )
                        else:
                            nc.scalar.copy(
                                otile[:, rb, mt * 512:(mt + 1) * 512], acc[:]
                            )
                        ci += 1
                    nc.sync.dma_start(out=e_ext[q, :, rb], in_=otile[:, rb])
    nc.finalize()
    _CACHE["perq"] = nc
    return nc


# --------------------------------------------------------------------------
# dense fallback graph: E = Jmat @ Zoh  (always valid)
# --------------------------------------------------------------------------
def _build_dense_graph():
    if "dense" in _CACHE:
        return _CACHE["dense"]
    nc = bacc.Bacc(None, target_bir_lowering=False)
    jt_ext = nc.declare_dram_parameter(
        "jt", [NB, 128, NB, 128], mybir.dt.bfloat16, isOutput=False
    )
    zoh_ext = nc.declare_dram_parameter(
        "zoh", [128, NB, M_LOC], mybir.dt.bfloat16, isOutput=False
    )
    out_ext = nc.declare_dram_parameter(
        "out", [F, M_LOC], mybir.dt.float32, isOutput=True
    )

    with tile.TileContext(nc) as tc:
        with (
            tc.tile_pool(name="zpool", bufs=1) as zpool,
            tc.tile_pool(name="jpool", bufs=3) as jpool,
            tc.tile_pool(name="opool", bufs=4) as opool,
            tc.tile_pool(name="psum", bufs=4, space=bass.MemorySpace.PSUM) as ppool,
        ):
            ztile = zpool.tile([128, NB, M_LOC], mybir.dt.bfloat16)
            nc.sync.dma_start(out=ztile[:], in_=zoh_ext[:])

            for i in range(NB):
                jtile = jpool.tile([128, NB, 128], mybir.dt.bfloat16)
                nc.sync.dma_start(out=jtile[:], in_=jt_ext[i])
                for mt in range(M_LOC // 512):
                    acc = ppool.tile([128, 512], mybir.dt.float32)
                    for k in range(NB):
                        nc.tensor.matmul(
                            acc[:],
                            jtile[:, k, :],
                            ztile[:, k, mt * 512:(mt + 1) * 512],
                            start=(k == 0),
                            stop=(k == NB - 1),
                        )
                    otile = opool.tile([128, 512], mybir.dt.float32)
                    nc.vector.tensor_copy(otile[:], acc[:])
                    nc.sync.dma_start(
                        out=out_ext[i * 128:(i + 1) * 128, mt * 512:(mt + 1) * 512],
                        in_=otile[:],
                    )
    nc.finalize()
    _CACHE["dense"] = nc
    return nc


def _softmax(x, axis):
    x = x - x.max(axis=axis, keepdims=True)
    e = np.exp(x)
    return e / e.sum(axis=axis, keepdims=True)


def _epilogue(E, Zi, weights, reg):
    """E: (q, L, M) fp32; pl + reg."""
    mx = E.max(axis=0)
    lge = mx + np.log(np.sum(np.exp(E - mx[None]), axis=0))
    Ec = np.take_along_axis(E, Zi[None], axis=0)[0]
    pl = -float(np.sum(weights * np.sum(Ec - lge, axis=0)))
    return np.float32(pl + reg)


def _reg_exact(A, Vaa):
    """L2 reg from the exact per-q Atq (cheap on host, ~1.4 M elements)."""
    diag = Vaa[:, np.arange(Q_AA), np.arange(Q_AA)]
    Atq = np.einsum("hij,hq->qij", A, diag).astype(np.float32)
    Atq[:, np.arange(L), np.arange(L)] = 0.0
    return Atq, LAMBDA * float(np.sum(Atq.astype(np.float64) ** 2))


def _z_maps(Zi):
    Zf = Zi.astype(np.float16)                             # values 0..20 exact
    zs = []
    for c in range(N_CORES):
        zc = Zf[:, c * M_LOC:(c + 1) * M_LOC]              # (256, 1024)
        zs.append(np.ascontiguousarray(
            zc.reshape(2, 128, M_LOC).transpose(1, 0, 2)
        ))                                                 # (128, 2, 1024)
    return zs


def _fast_path(Atot, reg, Zi, weights):
    """Shared-Atot 20-plane device path + host plane-21 reconstruction."""
    Atot16 = Atot.astype(np.float16)
    at_np = np.ascontiguousarray(
        Atot16.T.reshape(2, 128, L).transpose(1, 0, 2)
    )                                                      # (128, 2, 256)
    in_maps = [{"at": at_np, "z": z} for z in _z_maps(Zi)]

    nc = _build_fast_graph()
    res = run_bass_kernel_spmd(nc, in_maps, list(range(N_CORES)))
    E = np.empty((Q_AA, L, M_TOT), np.float32)
    E[:N_PLANES] = np.concatenate(
        [
            np.asarray(res.results[c]["e"])
            .astype(np.float32)                            # (q, 128, 2, m)
            .transpose(0, 2, 1, 3)                         # (q, rb, 128, m)
            .reshape(N_PLANES, L, M_LOC)
            for c in range(N_CORES)
        ],
        axis=2,
    )
    # sum_q E_q == rowsum(Atot16) since the one-hot planes partition (j,m)
    rowsum = Atot16.astype(np.float32).sum(axis=1)         # (L,)
    E[N_PLANES] = rowsum[:, None] - E[:N_PLANES].sum(axis=0)
    return _epilogue(E, Zi, weights, reg)


def _perq_path(Atq, reg, Zi, weights):
    """Per-q 21-plane device path (diag of Vaa not uniform)."""
    AtT = Atq.transpose(0, 2, 1)                           # (q, j, r)
    at_np = np.ascontiguousarray(
        AtT.reshape(Q_AA, 2, 128, L).transpose(0, 2, 1, 3)
    ).astype(np.float16)                                   # (q, 128, 2, L)
    in_maps = [{"at": at_np, "z": z} for z in _z_maps(Zi)]

    nc = _build_perq_graph()
    res = run_bass_kernel_spmd(nc, in_maps, list(range(N_CORES)))
    E = np.concatenate(
        [
            np.asarray(res.results[c]["e"])
            .astype(np.float32)
            .transpose(0, 2, 1, 3)
            .reshape(Q_AA, L, M_LOC)
            for c in range(N_CORES)
        ],
        axis=2,
    )                                                      # (q, L, M)
    return _epilogue(E, Zi, weights, reg)


def _fast_path_host(Atq, reg, Zi, weights):
    """Numpy fallback of the per-q formulation (if device paths fail)."""
    E = np.empty((Q_AA, L, M_TOT), np.float32)
    for q in range(Q_AA):
        E[q] = Atq[q] @ (Zi == q).astype(np.float32)
    return _epilogue(E, Zi, weights, reg)


def _dense_path(A, Vaa, Zi, weights):
    J = (A.reshape(H, L * L).T @ Vaa.reshape(H, Q_AA * Q_AA)).reshape(
        L, L, Q_AA, Q_AA
    )
    J[np.arange(L), np.arange(L)] = 0.0
    reg = LAMBDA * float(np.sum(J.astype(np.float64) ** 2))

    Jmat = np.ascontiguousarray(J.transpose(0, 2, 1, 3).reshape(F, F))
    JT4 = np.ascontiguousarray(Jmat.T).reshape(NB, 128, NB, 128)
    jt_np = np.ascontiguousarray(JT4.transpose(2, 1, 0, 3)).astype(
        ml_dtypes.bfloat16
    )

    colidx = np.arange(L)[:, None] * Q_AA + Zi             # (L, M)
    in_maps = []
    for c in range(N_CORES):
        ci = colidx[:, c * M_LOC:(c + 1) * M_LOC]
        zfull = np.zeros((F, M_LOC), np.float32)
        zfull[ci, np.arange(M_LOC)[None, :]] = 1.0
        zoh_np = np.ascontiguousarray(
            zfull.reshape(NB, 128, M_LOC).transpose(1, 0, 2)
        ).astype(ml_dtypes.bfloat16)
        in_maps.append({"jt": jt_np, "zoh": zoh_np})

    try:
        nc = _build_dense_graph()
        res = run_bass_kernel_spmd(nc, in_maps, list(range(N_CORES)))
        E = np.concatenate(
            [np.asarray(res.results[c]["out"]).astype(np.float32)
             for c in range(N_CORES)], axis=1
        )
    except Exception:
        shards = []
        for c in range(N_CORES):
            ci = colidx[:, c * M_LOC:(c + 1) * M_LOC]
            zfull = np.zeros((F, M_LOC), np.float32)
            zfull[ci, np.arange(M_LOC)[None, :]] = 1.0
            shards.append(Jmat @ zfull)
        E = np.concatenate(shards, axis=1)

    E3 = np.ascontiguousarray(E.reshape(L, Q_AA, M_TOT).transpose(1, 0, 2))
    return _epilogue(E3, Zi, weights, reg)


def kernel(reps_matrix, Q, K, V_metric, Z, weights):
    reps_matrix = np.asarray(reps_matrix, np.float32)
    Q = np.asarray(Q, np.float32)
    K = np.asarray(K, np.float32)
    V_metric = np.asarray(V_metric, np.float32)
    Zi = np.asarray(Z).astype(np.int64)
    weights = np.asarray(weights, np.float32)

    # --- host prologue: attention map + RBF kernel ---
    scores = np.einsum("hid,hjd->hij", Q, K) / np.sqrt(np.float32(DK))
    probs = _softmax(scores, axis=-1)
    A = 0.5 * (probs + probs.transpose(0, 2, 1))           # (H, L, L)

    V1 = np.einsum("qd,hdv->hqv", reps_matrix, V_metric)   # (H, q, dv)
    gamma = 1.0 / V1.shape[1]
    sq = np.sum(V1 * V1, axis=-1)
    D2 = sq[:, :, None] + sq[:, None, :] - 2.0 * np.einsum("hqv,hav->hqa", V1, V1)
    Vaa = np.exp(-gamma * np.maximum(D2, 0.0))             # (H, q, q)

    # dropped-residual bound for the fast paths: |E_res| <= max_offdiag(Vaa)
    # * max row-sum of sum_h A[h]
    offmax = float((Vaa * (1.0 - np.eye(Q_AA, dtype=np.float32))[None]).max())
    rowsum = float(np.abs(A).sum(axis=0).sum(axis=1).max())
    diagdev = float(
        np.abs(Vaa[:, np.arange(Q_AA), np.arange(Q_AA)] - 1.0).max()
    )
    if offmax * rowsum < 1e-7:
        Atq, reg = _reg_exact(A, Vaa)
        try:
            if diagdev * rowsum < 1e-2:
                # Vaa ~ I exactly enough to share one Atot across planes
                Atot = A.sum(axis=0).astype(np.float32)
                Atot[np.arange(L), np.arange(L)] = 0.0
                return _fast_path(Atot, reg, Zi, weights)
            return _perq_path(Atq, reg, Zi, weights)
        except Exception:
            return _fast_path_host(Atq, reg, Zi, weights)
    return _dense_path(A, Vaa, Zi, weights)


# revision 4
# speedup vs baseline: 1.1990x; 1.1990x over previous
"""AttentionDCA pseudo-likelihood loss on 8 Trainium2 NeuronCores — fast path.

Mathematical structure exploited: with this problem's data distribution the
RBF kernel Vaa[h] = exp(-gamma*D2) is numerically an identity matrix
(off-diagonal entries < 1e-20, since D2 ~ 4096 and gamma = 1/21; diagonal
within 5e-5 of 1).  Writing Vaa[h] = I + R_h, the coupling tensor collapses:

    J[r,j,q,a] = Atot[r,j] * delta(q,a) + (residual, bounded by max R)
    Atot       = sum_h A[h],  diagonal (r==j) zeroed

so the energy tensor is 21 small matmuls instead of one 5376x5376 GEMM:

    E[q,r,m] = sum_j Atot[r,j] * [Z[j,m] == q]

and because sum_q [Z[j,m]==q] == 1, the planes satisfy
sum_q E[q,r,m] = rowsum(Atot)[r]: the device only computes 20 of the 21
planes and the host reconstructs the last one by subtraction.

Per core (M sharded 1024-per-core): 20 x (256x256)@(256x1024) fp16 matmuls
(~2.7 GFLOP, ~37 us of PE) with the one-hot RHS built on-device from Z by
DVE is_equal compares (4x 16-bit mode).  PSUM is drained fp32->fp16 split
5:3 across ACT:DVE; E returns in fp16 (10.5 MB); host does the cheap
logsumexp / gather / weighted-sum epilogue and the exact per-q L2 reg.

Tiered fallbacks, guarded by exact bounds on the dropped terms:
  1. shared-Atot 20-plane path   (needs off-diag Vaa ~ 0 AND uniform diag)
  2. per-q Atq 21-plane path     (needs off-diag Vaa ~ 0 only)
  3. dense J-matmul device path  (always valid; numpy as a last resort)
"""

import sys
import numpy as np

for p in ("/opt/trn_rl_repo", "/root/.axon_site/_ro/trn_rl_repo"):
    if p not in sys.path:
        sys.path.insert(0, p)

import ml_dtypes

import concourse.bass as bass
from concourse import bacc, mybir, tile
from concourse.bass_utils import run_bass_kernel_spmd

Q_AA = 21
H = 32
L = 256
DK = 32
M_TOT = 8192
N_CORES = 8
M_LOC = M_TOT // N_CORES          # 1024
F = L * Q_AA                      # 5376 flattened (pos, aa) dim
NB = F // 128                     # 42 blocks of 128
LAMBDA = 1e-3
N_PLANES = Q_AA - 1               # device computes 20, host rebuilds the 21st

_CACHE = {}


# byte pattern of fp8(1.0) for the one-hot synthesis trick below
_FP8_DT = np.dtype(mybir.dt.np(mybir.dt.float8e4))
_FP8_ONE_BYTE = int(np.array(1.0, _FP8_DT).view(np.uint8))


# --------------------------------------------------------------------------
# fast graph (shared Atot, fp8 DoubleRow): E[q,r,m] = sum_j Atot[r,j] *
# [Z[j,m]==q], planes q = 0..19 only; plane 20 reconstructed on host.
#
# The one-hot is built as uint16 words (z==q)*0x38: the LOW byte of each
# word is fp8(1.0)/fp8(0.0), so a stride-2 fp8 bitcast view of the uint16
# tile IS the fp8 one-hot plane — produced by a single 4x-mode DVE op.
# With both operands fp8, MatmulPerfMode.DoubleRow contracts K=256 in one
# instruction at 0.5 cycles/row (PE ~10 us instead of ~39).
# --------------------------------------------------------------------------
def _build_fast_graph():
    if "fast" in _CACHE:
        return _CACHE["fast"]
    nc = bacc.Bacc(None, target_bir_lowering=False)
    # lhsT: at[k(128), kb, r] fp8 = Atot.T[kb*128+k, r] (one 64 KB tile
    # shared by every plane; DoubleRow reads it as weights[p, two=kb, f=r])
    at_ext = nc.declare_dram_parameter(
        "at", [128, 2, L], mybir.dt.float8e4, isOutput=False
    )
    # z[k(128), kb, m] fp16 = Z[kb*128+k, m] as float
    z_ext = nc.declare_dram_parameter(
        "z", [128, 2, M_LOC], mybir.dt.float16, isOutput=False
    )
    # DRAM layout matches the SBUF tile's (partition-major) element order:
    # e[q, k, rb, m] with output row r = rb*128 + k
    e_ext = nc.declare_dram_parameter(
        "e", [N_PLANES, 128, 2, M_LOC], mybir.dt.float16, isOutput=True
    )

    with tile.TileContext(nc) as tc:
        with (
            tc.tile_pool(name="const", bufs=1) as cpool,
            tc.tile_pool(name="out", bufs=6) as opool,
            tc.tile_pool(name="psum", bufs=4, space=bass.MemorySpace.PSUM) as ppool,
        ):
            zt = cpool.tile([128, 2, M_LOC], mybir.dt.float16)
            at = cpool.tile([128, 2, L], mybir.dt.float8e4)
            # all one-hot planes stay resident as uint16; synthesis runs a
            # few planes ahead of the matmul stream on DVE
            zq = cpool.tile([128, N_PLANES, 2, M_LOC], mybir.dt.uint16)

            nc.sync.dma_start(out=zt[:], in_=z_ext[:])
            nc.sync.dma_start(out=at[:], in_=at_ext[:])

            def _cmp(q):
                # (z == q) -> 1/0 uint16, * 0x38 -> low byte = fp8(1.0)
                nc.vector.tensor_scalar(
                    zq[:, q], zt[:], float(q), _FP8_ONE_BYTE,
                    mybir.AluOpType.is_equal, mybir.AluOpType.mult,
                )

            lookahead = 3
            for q in range(min(lookahead, N_PLANES)):
                _cmp(q)

            ci = 0
            for q in range(N_PLANES):
                if q + lookahead < N_PLANES:
                    _cmp(q + lookahead)
                otile = opool.tile([128, 2, M_LOC], mybir.dt.float16)
                # stride-2 fp8 view of the uint16 words = the one-hot plane
                zq8 = zq[:, q].bitcast(mybir.dt.float8e4)  # [128, 2, 2048]
                for rb in range(2):
                    acc = ppool.tile([128, 1024], mybir.dt.float32)
                    for mt in range(M_LOC // 512):
                        rhs = zq8[:, :, mt * 1024:(mt + 1) * 1024:2]
                        nc.tensor.matmul(
                            acc[:, mt * 512:(mt + 1) * 512],
                            at[:, :, rb * 128:(rb + 1) * 128],
                            rhs,
                            start=True,
                            stop=True,
                            perf_mode=mybir.MatmulPerfMode.DoubleRow,
                        )
                    # one merged [128,1024] PSUM->SBUF drain per rb, split
                    # 2:1 ACT:DVE (DVE also synthesizes the one-hots;
                    # gpsimd cannot read PSUM here)
                    if ci % 3 == 1:
                        nc.vector.tensor_copy(otile[:, rb], acc[:])
                    else:
                        nc.scalar.copy(otile[:, rb], acc[:])
                    ci += 1
                    # per-rb output DMAs pipeline the writeback finer and
                    # shrink the final drain after the last matmul
                    nc.sync.dma_start(out=e_ext[q, :, rb], in_=otile[:, rb])
    nc.finalize()   # Bacc.compile(): sync legalization + reg allocation
    _CACHE["fast"] = nc
    return nc


# --------------------------------------------------------------------------
# per-q graph: E[q,r,m] = sum_j Atq[q][r,j] * [Z[j,m]==q], all 21 planes
# (used when the Vaa diagonal is not uniform enough for the shared-Atot
# shortcut)
# --------------------------------------------------------------------------
def _build_perq_graph():
    if "perq" in _CACHE:
        return _CACHE["perq"]
    nc = bacc.Bacc(None, target_bir_lowering=False)
    at_ext = nc.declare_dram_parameter(
        "at", [Q_AA, 128, 2, L], mybir.dt.float16, isOutput=False
    )
    z_ext = nc.declare_dram_parameter(
        "z", [128, 2, M_LOC], mybir.dt.float16, isOutput=False
    )
    e_ext = nc.declare_dram_parameter(
        "e", [Q_AA, 128, 2, M_LOC], mybir.dt.float16, isOutput=True
    )

    with tile.TileContext(nc) as tc:
        with (
            tc.tile_pool(name="const", bufs=1) as cpool,
            tc.tile_pool(name="out", bufs=6) as opool,
            tc.tile_pool(name="psum", bufs=8, space=bass.MemorySpace.PSUM) as ppool,
        ):
            zt = cpool.tile([128, 2, M_LOC], mybir.dt.float16)
            at = cpool.tile([128, Q_AA, 2, L], mybir.dt.float16)
            zq = cpool.tile([128, Q_AA, 2, M_LOC], mybir.dt.float16)

            nc.sync.dma_start(out=zt[:], in_=z_ext[:])
            for q in range(Q_AA):
                nc.sync.dma_start(out=at[:, q], in_=at_ext[q])

            def _cmp(q):
                nc.vector.tensor_scalar(
                    zq[:, q], zt[:], float(q), None, mybir.AluOpType.is_equal
                )

            lookahead = 4
            for q in range(min(lookahead, Q_AA)):
                _cmp(q)

            ci = 0
            for q in range(Q_AA):
                if q + lookahead < Q_AA:
                    _cmp(q + lookahead)
                otile = opool.tile([128, 2, M_LOC], mybir.dt.float16)
                for rb in range(2):
                    for mt in range(M_LOC // 512):
                        acc = ppool.tile([128, 512], mybir.dt.float32)
                        for kb in range(2):
                            nc.tensor.matmul(
                                acc[:],
                                at[:, q, kb, rb * 128:(rb + 1) * 128],
                                zq[:, q, kb, mt * 512:(mt + 1) * 512],
                                start=(kb == 0),
                                stop=(kb == 1),
                            )
                        if ci % 3 == 2:
                            nc.vector.tensor_copy(
                                otile[:, rb, mt * 512:(mt + 1) * 512], acc[:]
                            )
                        else:
                            nc.scalar.copy(
                                otile[:, rb, mt * 512:(mt + 1) * 512], acc[:]
                            )
                        ci += 1
                    nc.sync.dma_start(out=e_ext[q, :, rb], in_=otile[:, rb])
    nc.finalize()
    _CACHE["perq"] = nc
    return nc


# --------------------------------------------------------------------------
# dense fallback graph: E = Jmat @ Zoh  (always valid)
# --------------------------------------------------------------------------
def _build_dense_graph():
    if "dense" in _CACHE:
        return _CACHE["dense"]
    nc = bacc.Bacc(None, target_bir_lowering=False)
    jt_ext = nc.declare_dram_parameter(
        "jt", [NB, 128, NB, 128], mybir.dt.bfloat16, isOutput=False
    )
    zoh_ext = nc.declare_dram_parameter(
        "zoh", [128, NB, M_LOC], mybir.dt.bfloat16, isOutput=False
    )
    out_ext = nc.declare_dram_parameter(
        "out", [F, M_LOC], mybir.dt.float32, isOutput=True
    )

    with tile.TileContext(nc) as tc:
        with (
            tc.tile_pool(name="zpool", bufs=1) as zpool,
            tc.tile_pool(name="jpool", bufs=3) as jpool,
            tc.tile_pool(name="opool", bufs=4) as opool,
            tc.tile_pool(name="psum", bufs=4, space=bass.MemorySpace.PSUM) as ppool,
        ):
            ztile = zpool.tile([128, NB, M_LOC], mybir.dt.bfloat16)
            nc.sync.dma_start(out=ztile[:], in_=zoh_ext[:])

            for i in range(NB):
                jtile = jpool.tile([128, NB, 128], mybir.dt.bfloat16)
                nc.sync.dma_start(out=jtile[:], in_=jt_ext[i])
                for mt in range(M_LOC // 512):
                    acc = ppool.tile([128, 512], mybir.dt.float32)
                    for k in range(NB):
                        nc.tensor.matmul(
                            acc[:],
                            jtile[:, k, :],
                            ztile[:, k, mt * 512:(mt + 1) * 512],
                            start=(k == 0),
                            stop=(k == NB - 1),
                        )
                    otile = opool.tile([128, 512], mybir.dt.float32)
                    nc.vector.tensor_copy(otile[:], acc[:])
                    nc.sync.dma_start(
                        out=out_ext[i * 128:(i + 1) * 128, mt * 512:(mt + 1) * 512],
                        in_=otile[:],
                    )
    nc.finalize()
    _CACHE["dense"] = nc
    return nc


def _softmax(x, axis):
    x = x - x.max(axis=axis, keepdims=True)
    e = np.exp(x)
    return e / e.sum(axis=axis, keepdims=True)


def _epilogue(E, Zi, weights, reg):
    """E: (q, L, M) fp32; pl + reg."""
    mx = E.max(axis=0)
    lge = mx + np.log(np.sum(np.exp(E - mx[None]), axis=0))
    Ec = np.take_along_axis(E, Zi[None], axis=0)[0]
    pl = -float(np.sum(weights * np.sum(Ec - lge, axis=0)))
    return np.float32(pl + reg)


def _reg_exact(A, Vaa):
    """L2 reg from the exact per-q Atq (cheap on host, ~1.4 M elements)."""
    diag = Vaa[:, np.arange(Q_AA), np.arange(Q_AA)]
    Atq = np.einsum("hij,hq->qij", A, diag).astype(np.float32)
    Atq[:, np.arange(L), np.arange(L)] = 0.0
    return Atq, LAMBDA * float(np.sum(Atq.astype(np.float64) ** 2))


def _z_maps(Zi):
    Zf = Zi.astype(np.float16)                             # values 0..20 exact
    zs = []
    for c in range(N_CORES):
        zc = Zf[:, c * M_LOC:(c + 1) * M_LOC]              # (256, 1024)
        zs.append(np.ascontiguousarray(
            zc.reshape(2, 128, M_LOC).transpose(1, 0, 2)
        ))                                                 # (128, 2, 1024)
    return zs


def _fast_path(Atot, reg, Zi, weights):
    """Shared-Atot fp8 20-plane device path + host plane-21 reconstruction."""
    Atot8 = Atot.astype(_FP8_DT)
    at_np = np.ascontiguousarray(
        Atot8.T.reshape(2, 128, L).transpose(1, 0, 2)
    )                                                      # (128, 2, 256) fp8
    in_maps = [{"at": at_np, "z": z} for z in _z_maps(Zi)]

    nc = _build_fast_graph()
    res = run_bass_kernel_spmd(nc, in_maps, list(range(N_CORES)))
    E = np.empty((Q_AA, L, M_TOT), np.float32)
    E[:N_PLANES] = np.concatenate(
        [
            np.asarray(res.results[c]["e"])
            .astype(np.float32)                            # (q, 128, 2, m)
            .transpose(0, 2, 1, 3)                         # (q, rb, 128, m)
            .reshape(N_PLANES, L, M_LOC)
            for c in range(N_CORES)
        ],
        axis=2,
    )
    # sum_q E_q == rowsum(fp8(Atot)) since the one-hot planes partition (j,m)
    rowsum = Atot8.astype(np.float32).sum(axis=1)          # (L,)
    E[N_PLANES] = rowsum[:, None] - E[:N_PLANES].sum(axis=0)
    return _epilogue(E, Zi, weights, reg)


def _perq_path(Atq, reg, Zi, weights):
    """Per-q 21-plane device path (diag of Vaa not uniform)."""
    AtT = Atq.transpose(0, 2, 1)                           # (q, j, r)
    at_np = np.ascontiguousarray(
        AtT.reshape(Q_AA, 2, 128, L).transpose(0, 2, 1, 3)
    ).astype(np.float16)                                   # (q, 128, 2, L)
    in_maps = [{"at": at_np, "z": z} for z in _z_maps(Zi)]

    nc = _build_perq_graph()
    res = run_bass_kernel_spmd(nc, in_maps, list(range(N_CORES)))
    E = np.concatenate(
        [
            np.asarray(res.results[c]["e"])
            .astype(np.float32)
            .transpose(0, 2, 1, 3)
            .reshape(Q_AA, L, M_LOC)
            for c in range(N_CORES)
        ],
        axis=2,
    )                                                      # (q, L, M)
    return _epilogue(E, Zi, weights, reg)


def _fast_path_host(Atq, reg, Zi, weights):
    """Numpy fallback of the per-q formulation (if device paths fail)."""
    E = np.empty((Q_AA, L, M_TOT), np.float32)
    for q in range(Q_AA):
        E[q] = Atq[q] @ (Zi == q).astype(np.float32)
    return _epilogue(E, Zi, weights, reg)


def _dense_path(A, Vaa, Zi, weights):
    J = (A.reshape(H, L * L).T @ Vaa.reshape(H, Q_AA * Q_AA)).reshape(
        L, L, Q_AA, Q_AA
    )
    J[np.arange(L), np.arange(L)] = 0.0
    reg = LAMBDA * float(np.sum(J.astype(np.float64) ** 2))

    Jmat = np.ascontiguousarray(J.transpose(0, 2, 1, 3).reshape(F, F))
    JT4 = np.ascontiguousarray(Jmat.T).reshape(NB, 128, NB, 128)
    jt_np = np.ascontiguousarray(JT4.transpose(2, 1, 0, 3)).astype(
        ml_dtypes.bfloat16
    )

    colidx = np.arange(L)[:, None] * Q_AA + Zi             # (L, M)
    in_maps = []
    for c in range(N_CORES):
        ci = colidx[:, c * M_LOC:(c + 1) * M_LOC]
        zfull = np.zeros((F, M_LOC), np.float32)
        zfull[ci, np.arange(M_LOC)[None, :]] = 1.0
        zoh_np = np.ascontiguousarray(
            zfull.reshape(NB, 128, M_LOC).transpose(1, 0, 2)
        ).astype(ml_dtypes.bfloat16)
        in_maps.append({"jt": jt_np, "zoh": zoh_np})

    try:
        nc = _build_dense_graph()
        res = run_bass_kernel_spmd(nc, in_maps, list(range(N_CORES)))
        E = np.concatenate(
            [np.asarray(res.results[c]["out"]).astype(np.float32)
             for c in range(N_CORES)], axis=1
        )
    except Exception:
        shards = []
        for c in range(N_CORES):
            ci = colidx[:, c * M_LOC:(c + 1) * M_LOC]
            zfull = np.zeros((F, M_LOC), np.float32)
            zfull[ci, np.arange(M_LOC)[None, :]] = 1.0
            shards.append(Jmat @ zfull)
        E = np.concatenate(shards, axis=1)

    E3 = np.ascontiguousarray(E.reshape(L, Q_AA, M_TOT).transpose(1, 0, 2))
    return _epilogue(E3, Zi, weights, reg)


def kernel(reps_matrix, Q, K, V_metric, Z, weights):
    reps_matrix = np.asarray(reps_matrix, np.float32)
    Q = np.asarray(Q, np.float32)
    K = np.asarray(K, np.float32)
    V_metric = np.asarray(V_metric, np.float32)
    Zi = np.asarray(Z).astype(np.int64)
    weights = np.asarray(weights, np.float32)

    # --- host prologue: attention map + RBF kernel ---
    scores = np.einsum("hid,hjd->hij", Q, K) / np.sqrt(np.float32(DK))
    probs = _softmax(scores, axis=-1)
    A = 0.5 * (probs + probs.transpose(0, 2, 1))           # (H, L, L)

    V1 = np.einsum("qd,hdv->hqv", reps_matrix, V_metric)   # (H, q, dv)
    gamma = 1.0 / V1.shape[1]
    sq = np.sum(V1 * V1, axis=-1)
    D2 = sq[:, :, None] + sq[:, None, :] - 2.0 * np.einsum("hqv,hav->hqa", V1, V1)
    Vaa = np.exp(-gamma * np.maximum(D2, 0.0))             # (H, q, q)

    # dropped-residual bound for the fast paths: |E_res| <= max_offdiag(Vaa)
    # * max row-sum of sum_h A[h]
    offmax = float((Vaa * (1.0 - np.eye(Q_AA, dtype=np.float32))[None]).max())
    rowsum = float(np.abs(A).sum(axis=0).sum(axis=1).max())
    diagdev = float(
        np.abs(Vaa[:, np.arange(Q_AA), np.arange(Q_AA)] - 1.0).max()
    )
    if offmax * rowsum < 1e-7:
        Atq, reg = _reg_exact(A, Vaa)
        try:
            if diagdev * rowsum < 1e-2:
                # Vaa ~ I exactly enough to share one Atot across planes
                Atot = A.sum(axis=0).astype(np.float32)
                Atot[np.arange(L), np.arange(L)] = 0.0
                return _fast_path(Atot, reg, Zi, weights)
            return _perq_path(Atq, reg, Zi, weights)
        except Exception:
            return _fast_path_host(Atq, reg, Zi, weights)
    return _dense_path(A, Vaa, Zi, weights)


# revision 5
# speedup vs baseline: 1.2453x; 1.0387x over previous
"""AttentionDCA pseudo-likelihood loss on 8 Trainium2 NeuronCores — fast path.

Mathematical structure exploited: with this problem's data distribution the
RBF kernel Vaa[h] = exp(-gamma*D2) is numerically an identity matrix
(off-diagonal entries < 1e-20, since D2 ~ 4096 and gamma = 1/21; diagonal
within 5e-5 of 1).  Writing Vaa[h] = I + R_h, the coupling tensor collapses:

    J[r,j,q,a] = Atot[r,j] * delta(q,a) + (residual, bounded by max R)
    Atot       = sum_h A[h],  diagonal (r==j) zeroed

so the energy tensor is 21 small matmuls instead of one 5376x5376 GEMM:

    E[q,r,m] = sum_j Atot[r,j] * [Z[j,m] == q]

and because sum_q [Z[j,m]==q] == 1, the planes satisfy
sum_q E[q,r,m] = rowsum(Atot)[r]: the device only computes 20 of the 21
planes and the host reconstructs the last one by subtraction.

Per core (M sharded 1024-per-core): 20 x (256x256)@(256x1024) fp16 matmuls
(~2.7 GFLOP, ~37 us of PE) with the one-hot RHS built on-device from Z by
DVE is_equal compares (4x 16-bit mode).  PSUM is drained fp32->fp16 split
5:3 across ACT:DVE; E returns in fp16 (10.5 MB); host does the cheap
logsumexp / gather / weighted-sum epilogue and the exact per-q L2 reg.

Tiered fallbacks, guarded by exact bounds on the dropped terms:
  1. shared-Atot 20-plane path   (needs off-diag Vaa ~ 0 AND uniform diag)
  2. per-q Atq 21-plane path     (needs off-diag Vaa ~ 0 only)
  3. dense J-matmul device path  (always valid; numpy as a last resort)
"""

import sys
import numpy as np

for p in ("/opt/trn_rl_repo", "/root/.axon_site/_ro/trn_rl_repo"):
    if p not in sys.path:
        sys.path.insert(0, p)

import ml_dtypes

import concourse.bass as bass
from concourse import bacc, mybir, tile
from concourse.bass_utils import run_bass_kernel_spmd

Q_AA = 21
H = 32
L = 256
DK = 32
M_TOT = 8192
N_CORES = 8
M_LOC = M_TOT // N_CORES          # 1024
F = L * Q_AA                      # 5376 flattened (pos, aa) dim
NB = F // 128                     # 42 blocks of 128
LAMBDA = 1e-3
N_PLANES = Q_AA - 1               # device computes 20, host rebuilds the 21st

_CACHE = {}


# byte pattern of fp8(1.0) for the one-hot synthesis trick below
_FP8_DT = np.dtype(mybir.dt.np(mybir.dt.float8e4))
_FP8_ONE_BYTE = int(np.array(1.0, _FP8_DT).view(np.uint8))
# uint8 output quantization scale; E in [0, rowsum_max], needs
# rowsum_max * QSCALE < 255 (guarded in kernel(); rowsum_max ~ 34 here)
QSCALE = 6.0


# --------------------------------------------------------------------------
# fast graph (shared Atot, fp8 DoubleRow): E[q,r,m] = sum_j Atot[r,j] *
# [Z[j,m]==q], planes q = 0..19 only; plane 20 reconstructed on host.
#
# The one-hot is built as uint16 words (z==q)*0x38: the LOW byte of each
# word is fp8(1.0)/fp8(0.0), so a stride-2 fp8 bitcast view of the uint16
# tile IS the fp8 one-hot plane — produced by a single 4x-mode DVE op.
# With both operands fp8, MatmulPerfMode.DoubleRow contracts K=256 in one
# instruction at 0.5 cycles/row (PE ~10 us instead of ~39).
# --------------------------------------------------------------------------
def _build_fast_graph():
    if "fast" in _CACHE:
        return _CACHE["fast"]
    nc = bacc.Bacc(None, target_bir_lowering=False)
    # lhsT: at[k(128), kb, r] fp8 = Atot.T[kb*128+k, r] (one 64 KB tile
    # shared by every plane; DoubleRow reads it as weights[p, two=kb, f=r])
    at_ext = nc.declare_dram_parameter(
        "at", [128, 2, L], mybir.dt.float8e4, isOutput=False
    )
    # z[k(128), kb, m] fp16 = Z[kb*128+k, m] as float
    z_ext = nc.declare_dram_parameter(
        "z", [128, 2, M_LOC], mybir.dt.float16, isOutput=False
    )
    # DRAM layout matches the SBUF tile's (partition-major) element order:
    # e[q, k, rb, m] with output row r = rb*128 + k
    # output quantized uint8: E*QSCALE rounded-to-nearest-even (verified
    # device semantics, saturating). Halves the dominant output DMA bytes.
    e_ext = nc.declare_dram_parameter(
        "e", [N_PLANES, 128, 2, M_LOC], mybir.dt.uint8, isOutput=True
    )

    with tile.TileContext(nc) as tc:
        with (
            tc.tile_pool(name="const", bufs=1) as cpool,
            tc.tile_pool(name="out", bufs=6) as opool,
            tc.tile_pool(name="psum", bufs=4, space=bass.MemorySpace.PSUM) as ppool,
        ):
            zt = cpool.tile([128, 2, M_LOC], mybir.dt.float16)
            at = cpool.tile([128, 2, L], mybir.dt.float8e4)
            # all one-hot planes stay resident as uint16; synthesis runs a
            # few planes ahead of the matmul stream on DVE
            zq = cpool.tile([128, N_PLANES, 2, M_LOC], mybir.dt.uint16)

            nc.sync.dma_start(out=zt[:], in_=z_ext[:])
            nc.sync.dma_start(out=at[:], in_=at_ext[:])

            def _cmp(q):
                # (z == q) -> 1/0 uint16, * 0x38 -> low byte = fp8(1.0)
                nc.vector.tensor_scalar(
                    zq[:, q], zt[:], float(q), _FP8_ONE_BYTE,
                    mybir.AluOpType.is_equal, mybir.AluOpType.mult,
                )

            lookahead = 3
            for q in range(min(lookahead, N_PLANES)):
                _cmp(q)

            ci = 0
            for q in range(N_PLANES):
                if q + lookahead < N_PLANES:
                    _cmp(q + lookahead)
                otile = opool.tile([128, 2, M_LOC], mybir.dt.uint8)
                # stride-2 fp8 view of the uint16 words = the one-hot plane
                zq8 = zq[:, q].bitcast(mybir.dt.float8e4)  # [128, 2, 2048]
                for rb in range(2):
                    acc = ppool.tile([128, 1024], mybir.dt.float32)
                    for mt in range(M_LOC // 512):
                        rhs = zq8[:, :, mt * 1024:(mt + 1) * 1024:2]
                        nc.tensor.matmul(
                            acc[:, mt * 512:(mt + 1) * 512],
                            at[:, :, rb * 128:(rb + 1) * 128],
                            rhs,
                            start=True,
                            stop=True,
                            perf_mode=mybir.MatmulPerfMode.DoubleRow,
                        )
                    # one merged [128,1024] PSUM->SBUF drain per rb that
                    # also quantizes E*QSCALE -> uint8, split 2:1 ACT:DVE
                    # (DVE also synthesizes the one-hots; gpsimd cannot
                    # read PSUM here)
                    if ci % 3 == 1:
                        nc.vector.tensor_scalar(
                            otile[:, rb], acc[:], QSCALE, None,
                            mybir.AluOpType.mult,
                        )
                    else:
                        nc.scalar.activation(
                            otile[:, rb], acc[:],
                            mybir.ActivationFunctionType.Copy, 0.0, QSCALE,
                        )
                    ci += 1
                    # per-rb output DMAs pipeline the writeback finer and
                    # shrink the final drain after the last matmul
                    nc.sync.dma_start(out=e_ext[q, :, rb], in_=otile[:, rb])
    nc.finalize()   # Bacc.compile(): sync legalization + reg allocation
    _CACHE["fast"] = nc
    return nc


# --------------------------------------------------------------------------
# per-q graph: E[q,r,m] = sum_j Atq[q][r,j] * [Z[j,m]==q], all 21 planes
# (used when the Vaa diagonal is not uniform enough for the shared-Atot
# shortcut)
# --------------------------------------------------------------------------
def _build_perq_graph():
    if "perq" in _CACHE:
        return _CACHE["perq"]
    nc = bacc.Bacc(None, target_bir_lowering=False)
    at_ext = nc.declare_dram_parameter(
        "at", [Q_AA, 128, 2, L], mybir.dt.float16, isOutput=False
    )
    z_ext = nc.declare_dram_parameter(
        "z", [128, 2, M_LOC], mybir.dt.float16, isOutput=False
    )
    e_ext = nc.declare_dram_parameter(
        "e", [Q_AA, 128, 2, M_LOC], mybir.dt.float16, isOutput=True
    )

    with tile.TileContext(nc) as tc:
        with (
            tc.tile_pool(name="const", bufs=1) as cpool,
            tc.tile_pool(name="out", bufs=6) as opool,
            tc.tile_pool(name="psum", bufs=8, space=bass.MemorySpace.PSUM) as ppool,
        ):
            zt = cpool.tile([128, 2, M_LOC], mybir.dt.float16)
            at = cpool.tile([128, Q_AA, 2, L], mybir.dt.float16)
            zq = cpool.tile([128, Q_AA, 2, M_LOC], mybir.dt.float16)

            nc.sync.dma_start(out=zt[:], in_=z_ext[:])
            for q in range(Q_AA):
                nc.sync.dma_start(out=at[:, q], in_=at_ext[q])

            def _cmp(q):
                nc.vector.tensor_scalar(
                    zq[:, q], zt[:], float(q), None, mybir.AluOpType.is_equal
                )

            lookahead = 4
            for q in range(min(lookahead, Q_AA)):
                _cmp(q)

            ci = 0
            for q in range(Q_AA):
                if q + lookahead < Q_AA:
                    _cmp(q + lookahead)
                otile = opool.tile([128, 2, M_LOC], mybir.dt.float16)
                for rb in range(2):
                    for mt in range(M_LOC // 512):
                        acc = ppool.tile([128, 512], mybir.dt.float32)
                        for kb in range(2):
                            nc.tensor.matmul(
                                acc[:],
                                at[:, q, kb, rb * 128:(rb + 1) * 128],
                                zq[:, q, kb, mt * 512:(mt + 1) * 512],
                                start=(kb == 0),
                                stop=(kb == 1),
                            )
                        if ci % 3 == 2:
                            nc.vector.tensor_copy(
                                otile[:, rb, mt * 512:(mt + 1) * 512], acc[:]
                            )
                        else:
                            nc.scalar.copy(
                                otile[:, rb, mt * 512:(mt + 1) * 512], acc[:]
                            )
                        ci += 1
                    nc.sync.dma_start(out=e_ext[q, :, rb], in_=otile[:, rb])
    nc.finalize()
    _CACHE["perq"] = nc
    return nc


# --------------------------------------------------------------------------
# dense fallback graph: E = Jmat @ Zoh  (always valid)
# --------------------------------------------------------------------------
def _build_dense_graph():
    if "dense" in _CACHE:
        return _CACHE["dense"]
    nc = bacc.Bacc(None, target_bir_lowering=False)
    jt_ext = nc.declare_dram_parameter(
        "jt", [NB, 128, NB, 128], mybir.dt.bfloat16, isOutput=False
    )
    zoh_ext = nc.declare_dram_parameter(
        "zoh", [128, NB, M_LOC], mybir.dt.bfloat16, isOutput=False
    )
    out_ext = nc.declare_dram_parameter(
        "out", [F, M_LOC], mybir.dt.float32, isOutput=True
    )

    with tile.TileContext(nc) as tc:
        with (
            tc.tile_pool(name="zpool", bufs=1) as zpool,
            tc.tile_pool(name="jpool", bufs=3) as jpool,
            tc.tile_pool(name="opool", bufs=4) as opool,
            tc.tile_pool(name="psum", bufs=4, space=bass.MemorySpace.PSUM) as ppool,
        ):
            ztile = zpool.tile([128, NB, M_LOC], mybir.dt.bfloat16)
            nc.sync.dma_start(out=ztile[:], in_=zoh_ext[:])

            for i in range(NB):
                jtile = jpool.tile([128, NB, 128], mybir.dt.bfloat16)
                nc.sync.dma_start(out=jtile[:], in_=jt_ext[i])
                for mt in range(M_LOC // 512):
                    acc = ppool.tile([128, 512], mybir.dt.float32)
                    for k in range(NB):
                        nc.tensor.matmul(
                            acc[:],
                            jtile[:, k, :],
                            ztile[:, k, mt * 512:(mt + 1) * 512],
                            start=(k == 0),
                            stop=(k == NB - 1),
                        )
                    otile = opool.tile([128, 512], mybir.dt.float32)
                    nc.vector.tensor_copy(otile[:], acc[:])
                    nc.sync.dma_start(
                        out=out_ext[i * 128:(i + 1) * 128, mt * 512:(mt + 1) * 512],
                        in_=otile[:],
                    )
    nc.finalize()
    _CACHE["dense"] = nc
    return nc


def _softmax(x, axis):
    x = x - x.max(axis=axis, keepdims=True)
    e = np.exp(x)
    return e / e.sum(axis=axis, keepdims=True)


def _epilogue(E, Zi, weights, reg):
    """E: (q, L, M) fp32; pl + reg."""
    mx = E.max(axis=0)
    lge = mx + np.log(np.sum(np.exp(E - mx[None]), axis=0))
    Ec = np.take_along_axis(E, Zi[None], axis=0)[0]
    pl = -float(np.sum(weights * np.sum(Ec - lge, axis=0)))
    return np.float32(pl + reg)


def _reg_exact(A, Vaa):
    """L2 reg from the exact per-q Atq (cheap on host, ~1.4 M elements)."""
    diag = Vaa[:, np.arange(Q_AA), np.arange(Q_AA)]
    Atq = np.einsum("hij,hq->qij", A, diag).astype(np.float32)
    Atq[:, np.arange(L), np.arange(L)] = 0.0
    return Atq, LAMBDA * float(np.sum(Atq.astype(np.float64) ** 2))


def _z_maps(Zi):
    Zf = Zi.astype(np.float16)                             # values 0..20 exact
    zs = []
    for c in range(N_CORES):
        zc = Zf[:, c * M_LOC:(c + 1) * M_LOC]              # (256, 1024)
        zs.append(np.ascontiguousarray(
            zc.reshape(2, 128, M_LOC).transpose(1, 0, 2)
        ))                                                 # (128, 2, 1024)
    return zs


def _fast_path(Atot, reg, Zi, weights):
    """Shared-Atot fp8 20-plane device path + host plane-21 reconstruction."""
    Atot8 = Atot.astype(_FP8_DT)
    at_np = np.ascontiguousarray(
        Atot8.T.reshape(2, 128, L).transpose(1, 0, 2)
    )                                                      # (128, 2, 256) fp8
    in_maps = [{"at": at_np, "z": z} for z in _z_maps(Zi)]

    nc = _build_fast_graph()
    res = run_bass_kernel_spmd(nc, in_maps, list(range(N_CORES)))
    E = np.empty((Q_AA, L, M_TOT), np.float32)
    E[:N_PLANES] = np.concatenate(
        [
            np.asarray(res.results[c]["e"])
            .astype(np.float32)                            # (q, 128, 2, m)
            .transpose(0, 2, 1, 3)                         # (q, rb, 128, m)
            .reshape(N_PLANES, L, M_LOC)
            for c in range(N_CORES)
        ],
        axis=2,
    ) * np.float32(1.0 / QSCALE)                           # dequantize
    # sum_q E_q == rowsum(fp8(Atot)) since the one-hot planes partition (j,m)
    rowsum = Atot8.astype(np.float32).sum(axis=1)          # (L,)
    E[N_PLANES] = rowsum[:, None] - E[:N_PLANES].sum(axis=0)
    return _epilogue(E, Zi, weights, reg)


def _perq_path(Atq, reg, Zi, weights):
    """Per-q 21-plane device path (diag of Vaa not uniform)."""
    AtT = Atq.transpose(0, 2, 1)                           # (q, j, r)
    at_np = np.ascontiguousarray(
        AtT.reshape(Q_AA, 2, 128, L).transpose(0, 2, 1, 3)
    ).astype(np.float16)                                   # (q, 128, 2, L)
    in_maps = [{"at": at_np, "z": z} for z in _z_maps(Zi)]

    nc = _build_perq_graph()
    res = run_bass_kernel_spmd(nc, in_maps, list(range(N_CORES)))
    E = np.concatenate(
        [
            np.asarray(res.results[c]["e"])
            .astype(np.float32)
            .transpose(0, 2, 1, 3)
            .reshape(Q_AA, L, M_LOC)
            for c in range(N_CORES)
        ],
        axis=2,
    )                                                      # (q, L, M)
    return _epilogue(E, Zi, weights, reg)


def _fast_path_host(Atq, reg, Zi, weights):
    """Numpy fallback of the per-q formulation (if device paths fail)."""
    E = np.empty((Q_AA, L, M_TOT), np.float32)
    for q in range(Q_AA):
        E[q] = Atq[q] @ (Zi == q).astype(np.float32)
    return _epilogue(E, Zi, weights, reg)


def _dense_path(A, Vaa, Zi, weights):
    J = (A.reshape(H, L * L).T @ Vaa.reshape(H, Q_AA * Q_AA)).reshape(
        L, L, Q_AA, Q_AA
    )
    J[np.arange(L), np.arange(L)] = 0.0
    reg = LAMBDA * float(np.sum(J.astype(np.float64) ** 2))

    Jmat = np.ascontiguousarray(J.transpose(0, 2, 1, 3).reshape(F, F))
    JT4 = np.ascontiguousarray(Jmat.T).reshape(NB, 128, NB, 128)
    jt_np = np.ascontiguousarray(JT4.transpose(2, 1, 0, 3)).astype(
        ml_dtypes.bfloat16
    )

    colidx = np.arange(L)[:, None] * Q_AA + Zi             # (L, M)
    in_maps = []
    for c in range(N_CORES):
        ci = colidx[:, c * M_LOC:(c + 1) * M_LOC]
        zfull = np.zeros((F, M_LOC), np.float32)
        zfull[ci, np.arange(M_LOC)[None, :]] = 1.0
        zoh_np = np.ascontiguousarray(
            zfull.reshape(NB, 128, M_LOC).transpose(1, 0, 2)
        ).astype(ml_dtypes.bfloat16)
        in_maps.append({"jt": jt_np, "zoh": zoh_np})

    try:
        nc = _build_dense_graph()
        res = run_bass_kernel_spmd(nc, in_maps, list(range(N_CORES)))
        E = np.concatenate(
            [np.asarray(res.results[c]["out"]).astype(np.float32)
             for c in range(N_CORES)], axis=1
        )
    except Exception:
        shards = []
        for c in range(N_CORES):
            ci = colidx[:, c * M_LOC:(c + 1) * M_LOC]
            zfull = np.zeros((F, M_LOC), np.float32)
            zfull[ci, np.arange(M_LOC)[None, :]] = 1.0
            shards.append(Jmat @ zfull)
        E = np.concatenate(shards, axis=1)

    E3 = np.ascontiguousarray(E.reshape(L, Q_AA, M_TOT).transpose(1, 0, 2))
    return _epilogue(E3, Zi, weights, reg)


def kernel(reps_matrix, Q, K, V_metric, Z, weights):
    reps_matrix = np.asarray(reps_matrix, np.float32)
    Q = np.asarray(Q, np.float32)
    K = np.asarray(K, np.float32)
    V_metric = np.asarray(V_metric, np.float32)
    Zi = np.asarray(Z).astype(np.int64)
    weights = np.asarray(weights, np.float32)

    # --- host prologue: attention map + RBF kernel ---
    scores = np.einsum("hid,hjd->hij", Q, K) / np.sqrt(np.float32(DK))
    probs = _softmax(scores, axis=-1)
    A = 0.5 * (probs + probs.transpose(0, 2, 1))           # (H, L, L)

    V1 = np.einsum("qd,hdv->hqv", reps_matrix, V_metric)   # (H, q, dv)
    gamma = 1.0 / V1.shape[1]
    sq = np.sum(V1 * V1, axis=-1)
    D2 = sq[:, :, None] + sq[:, None, :] - 2.0 * np.einsum("hqv,hav->hqa", V1, V1)
    Vaa = np.exp(-gamma * np.maximum(D2, 0.0))             # (H, q, q)

    # dropped-residual bound for the fast paths: |E_res| <= max_offdiag(Vaa)
    # * max row-sum of sum_h A[h]
    offmax = float((Vaa * (1.0 - np.eye(Q_AA, dtype=np.float32))[None]).max())
    rowsum = float(np.abs(A).sum(axis=0).sum(axis=1).max())
    diagdev = float(
        np.abs(Vaa[:, np.arange(Q_AA), np.arange(Q_AA)] - 1.0).max()
    )
    if offmax * rowsum < 1e-7:
        Atq, reg = _reg_exact(A, Vaa)
        try:
            Atot = A.sum(axis=0).astype(np.float32)
            Atot[np.arange(L), np.arange(L)] = 0.0
            rs_max = float(Atot.astype(_FP8_DT).astype(np.float32).sum(1).max())
            if diagdev * rowsum < 1e-2 and rs_max * QSCALE < 250.0:
                # Vaa ~ I enough to share one Atot, and E fits uint8 range
                return _fast_path(Atot, reg, Zi, weights)
            return _perq_path(Atq, reg, Zi, weights)
        except Exception:
            return _fast_path_host(Atq, reg, Zi, weights)
    return _dense_path(A, Vaa, Zi, weights)


# revision 6
# speedup vs baseline: 1.2925x; 1.0379x over previous
"""AttentionDCA pseudo-likelihood loss on 8 Trainium2 NeuronCores — fast path.

Mathematical structure exploited: with this problem's data distribution the
RBF kernel Vaa[h] = exp(-gamma*D2) is numerically an identity matrix
(off-diagonal entries < 1e-20, since D2 ~ 4096 and gamma = 1/21; diagonal
within 5e-5 of 1).  Writing Vaa[h] = I + R_h, the coupling tensor collapses:

    J[r,j,q,a] = Atot[r,j] * delta(q,a) + (residual, bounded by max R)
    Atot       = sum_h A[h],  diagonal (r==j) zeroed

so the energy tensor is 21 small matmuls instead of one 5376x5376 GEMM:

    E[q,r,m] = sum_j Atot[r,j] * [Z[j,m] == q]

and because sum_q [Z[j,m]==q] == 1, the planes satisfy
sum_q E[q,r,m] = rowsum(Atot)[r]: the device only computes 20 of the 21
planes and the host reconstructs the last one by subtraction.

Per core (M sharded 1024-per-core): 20 x (256x256)@(256x1024) fp16 matmuls
(~2.7 GFLOP, ~37 us of PE) with the one-hot RHS built on-device from Z by
DVE is_equal compares (4x 16-bit mode).  PSUM is drained fp32->fp16 split
5:3 across ACT:DVE; E returns in fp16 (10.5 MB); host does the cheap
logsumexp / gather / weighted-sum epilogue and the exact per-q L2 reg.

Tiered fallbacks, guarded by exact bounds on the dropped terms:
  1. shared-Atot 20-plane path   (needs off-diag Vaa ~ 0 AND uniform diag)
  2. per-q Atq 21-plane path     (needs off-diag Vaa ~ 0 only)
  3. dense J-matmul device path  (always valid; numpy as a last resort)
"""

import sys
import numpy as np

for p in ("/opt/trn_rl_repo", "/root/.axon_site/_ro/trn_rl_repo"):
    if p not in sys.path:
        sys.path.insert(0, p)

import ml_dtypes

import concourse.bass as bass
from concourse import bacc, mybir, tile
from concourse.bass_utils import run_bass_kernel_spmd

Q_AA = 21
H = 32
L = 256
DK = 32
M_TOT = 8192
N_CORES = 8
M_LOC = M_TOT // N_CORES          # 1024
F = L * Q_AA                      # 5376 flattened (pos, aa) dim
NB = F // 128                     # 42 blocks of 128
LAMBDA = 1e-3
N_PLANES = Q_AA - 1               # device computes 20, host rebuilds the 21st

_CACHE = {}


# byte pattern of fp8(1.0) for the one-hot synthesis trick below
_FP8_DT = np.dtype(mybir.dt.np(mybir.dt.float8e4))
_FP8_ONE_BYTE = int(np.array(1.0, _FP8_DT).view(np.uint8))
# uint8 output quantization scale; E in [0, rowsum_max], needs
# rowsum_max * QSCALE < 255 (guarded in kernel(); rowsum_max ~ 34 here)
QSCALE = 6.0


# --------------------------------------------------------------------------
# fast graph (shared Atot, fp8 DoubleRow): E[q,r,m] = sum_j Atot[r,j] *
# [Z[j,m]==q], planes q = 0..19 only; plane 20 reconstructed on host.
#
# The one-hot is built as uint16 words (z==q)*0x38: the LOW byte of each
# word is fp8(1.0)/fp8(0.0), so a stride-2 fp8 bitcast view of the uint16
# tile IS the fp8 one-hot plane — produced by a single 4x-mode DVE op.
# With both operands fp8, MatmulPerfMode.DoubleRow contracts K=256 in one
# instruction at 0.5 cycles/row (PE ~10 us instead of ~39).
# --------------------------------------------------------------------------
def _build_fast_graph():
    if "fast" in _CACHE:
        return _CACHE["fast"]
    nc = bacc.Bacc(None, target_bir_lowering=False)
    # lhsT: at[k(128), kb, r] fp8 = Atot.T[kb*128+k, r] (one 64 KB tile
    # shared by every plane; DoubleRow reads it as weights[p, two=kb, f=r])
    at_ext = nc.declare_dram_parameter(
        "at", [128, 2, L], mybir.dt.float8e4, isOutput=False
    )
    # z[k(128), kb, m] fp16 = Z[kb*128+k, m] as float
    z_ext = nc.declare_dram_parameter(
        "z", [128, 2, M_LOC], mybir.dt.float16, isOutput=False
    )
    # DRAM layout matches the SBUF tile's (partition-major) element order:
    # e[q, k, rb, m] with output row r = rb*128 + k
    # output quantized uint8: E*QSCALE rounded-to-nearest-even (verified
    # device semantics, saturating). Halves the dominant output DMA bytes.
    e_ext = nc.declare_dram_parameter(
        "e", [N_PLANES, 128, 2, M_LOC], mybir.dt.uint8, isOutput=True
    )

    with tile.TileContext(nc) as tc:
        with (
            tc.tile_pool(name="const", bufs=1) as cpool,
            tc.tile_pool(name="out", bufs=6) as opool,
            tc.tile_pool(name="psum", bufs=4, space=bass.MemorySpace.PSUM) as ppool,
        ):
            zt = cpool.tile([128, 2, M_LOC], mybir.dt.float16)
            at = cpool.tile([128, 2, L], mybir.dt.float8e4)
            # all one-hot planes stay resident as uint16; synthesis runs a
            # few planes ahead of the matmul stream on DVE
            zq = cpool.tile([128, N_PLANES, 2, M_LOC], mybir.dt.uint16)

            nc.sync.dma_start(out=zt[:], in_=z_ext[:])
            nc.sync.dma_start(out=at[:], in_=at_ext[:])

            def _cmp(q):
                # (z == q) -> 1/0 uint16, * 0x38 -> low byte = fp8(1.0)
                nc.vector.tensor_scalar(
                    zq[:, q], zt[:], float(q), _FP8_ONE_BYTE,
                    mybir.AluOpType.is_equal, mybir.AluOpType.mult,
                )

            lookahead = 3
            for q in range(min(lookahead, N_PLANES)):
                _cmp(q)

            ci = 0
            for q in range(N_PLANES):
                if q + lookahead < N_PLANES:
                    _cmp(q + lookahead)
                otile = opool.tile([128, 2, M_LOC], mybir.dt.uint8)
                # stride-2 fp8 view of the uint16 words = the one-hot plane
                zq8 = zq[:, q].bitcast(mybir.dt.float8e4)  # [128, 2, 2048]
                for rb in range(2):
                    acc = ppool.tile([128, 1024], mybir.dt.float32)
                    for mt in range(M_LOC // 512):
                        rhs = zq8[:, :, mt * 1024:(mt + 1) * 1024:2]
                        nc.tensor.matmul(
                            acc[:, mt * 512:(mt + 1) * 512],
                            at[:, :, rb * 128:(rb + 1) * 128],
                            rhs,
                            start=True,
                            stop=True,
                            perf_mode=mybir.MatmulPerfMode.DoubleRow,
                        )
                    # one merged [128,1024] PSUM->SBUF drain per rb that
                    # also quantizes E*QSCALE -> uint8, split 2:1 ACT:DVE
                    # (DVE also synthesizes the one-hots; gpsimd cannot
                    # read PSUM here)
                    if ci % 3 == 1:
                        nc.vector.tensor_scalar(
                            otile[:, rb], acc[:], QSCALE, None,
                            mybir.AluOpType.mult,
                        )
                    else:
                        nc.scalar.activation(
                            otile[:, rb], acc[:],
                            mybir.ActivationFunctionType.Copy, 0.0, QSCALE,
                        )
                    # per-rb output DMAs pipeline the writeback finer;
                    # alternate between the SP/HWDGE queue and the Pool/SWDGE
                    # queue (two parallel DMA issue paths — HWDGE generation
                    # at 625 ns/DMA otherwise congests the writeback tail)
                    if ci % 2 == 0:
                        nc.gpsimd.dma_start(out=e_ext[q, :, rb], in_=otile[:, rb])
                    else:
                        nc.sync.dma_start(out=e_ext[q, :, rb], in_=otile[:, rb])
                    ci += 1
    nc.finalize()   # Bacc.compile(): sync legalization + reg allocation
    _CACHE["fast"] = nc
    return nc


# --------------------------------------------------------------------------
# per-q graph: E[q,r,m] = sum_j Atq[q][r,j] * [Z[j,m]==q], all 21 planes
# (used when the Vaa diagonal is not uniform enough for the shared-Atot
# shortcut)
# --------------------------------------------------------------------------
def _build_perq_graph():
    if "perq" in _CACHE:
        return _CACHE["perq"]
    nc = bacc.Bacc(None, target_bir_lowering=False)
    at_ext = nc.declare_dram_parameter(
        "at", [Q_AA, 128, 2, L], mybir.dt.float16, isOutput=False
    )
    z_ext = nc.declare_dram_parameter(
        "z", [128, 2, M_LOC], mybir.dt.float16, isOutput=False
    )
    e_ext = nc.declare_dram_parameter(
        "e", [Q_AA, 128, 2, M_LOC], mybir.dt.float16, isOutput=True
    )

    with tile.TileContext(nc) as tc:
        with (
            tc.tile_pool(name="const", bufs=1) as cpool,
            tc.tile_pool(name="out", bufs=6) as opool,
            tc.tile_pool(name="psum", bufs=8, space=bass.MemorySpace.PSUM) as ppool,
        ):
            zt = cpool.tile([128, 2, M_LOC], mybir.dt.float16)
            at = cpool.tile([128, Q_AA, 2, L], mybir.dt.float16)
            zq = cpool.tile([128, Q_AA, 2, M_LOC], mybir.dt.float16)

            nc.sync.dma_start(out=zt[:], in_=z_ext[:])
            for q in range(Q_AA):
                nc.sync.dma_start(out=at[:, q], in_=at_ext[q])

            def _cmp(q):
                nc.vector.tensor_scalar(
                    zq[:, q], zt[:], float(q), None, mybir.AluOpType.is_equal
                )

            lookahead = 4
            for q in range(min(lookahead, Q_AA)):
                _cmp(q)

            ci = 0
            for q in range(Q_AA):
                if q + lookahead < Q_AA:
                    _cmp(q + lookahead)
                otile = opool.tile([128, 2, M_LOC], mybir.dt.float16)
                for rb in range(2):
                    for mt in range(M_LOC // 512):
                        acc = ppool.tile([128, 512], mybir.dt.float32)
                        for kb in range(2):
                            nc.tensor.matmul(
                                acc[:],
                                at[:, q, kb, rb * 128:(rb + 1) * 128],
                                zq[:, q, kb, mt * 512:(mt + 1) * 512],
                                start=(kb == 0),
                                stop=(kb == 1),
                            )
                        if ci % 3 == 2:
                            nc.vector.tensor_copy(
                                otile[:, rb, mt * 512:(mt + 1) * 512], acc[:]
                            )
                        else:
                            nc.scalar.copy(
                                otile[:, rb, mt * 512:(mt + 1) * 512], acc[:]
                            )
                        ci += 1
                    nc.sync.dma_start(out=e_ext[q, :, rb], in_=otile[:, rb])
    nc.finalize()
    _CACHE["perq"] = nc
    return nc


# --------------------------------------------------------------------------
# dense fallback graph: E = Jmat @ Zoh  (always valid)
# --------------------------------------------------------------------------
def _build_dense_graph():
    if "dense" in _CACHE:
        return _CACHE["dense"]
    nc = bacc.Bacc(None, target_bir_lowering=False)
    jt_ext = nc.declare_dram_parameter(
        "jt", [NB, 128, NB, 128], mybir.dt.bfloat16, isOutput=False
    )
    zoh_ext = nc.declare_dram_parameter(
        "zoh", [128, NB, M_LOC], mybir.dt.bfloat16, isOutput=False
    )
    out_ext = nc.declare_dram_parameter(
        "out", [F, M_LOC], mybir.dt.float32, isOutput=True
    )

    with tile.TileContext(nc) as tc:
        with (
            tc.tile_pool(name="zpool", bufs=1) as zpool,
            tc.tile_pool(name="jpool", bufs=3) as jpool,
            tc.tile_pool(name="opool", bufs=4) as opool,
            tc.tile_pool(name="psum", bufs=4, space=bass.MemorySpace.PSUM) as ppool,
        ):
            ztile = zpool.tile([128, NB, M_LOC], mybir.dt.bfloat16)
            nc.sync.dma_start(out=ztile[:], in_=zoh_ext[:])

            for i in range(NB):
                jtile = jpool.tile([128, NB, 128], mybir.dt.bfloat16)
                nc.sync.dma_start(out=jtile[:], in_=jt_ext[i])
                for mt in range(M_LOC // 512):
                    acc = ppool.tile([128, 512], mybir.dt.float32)
                    for k in range(NB):
                        nc.tensor.matmul(
                            acc[:],
                            jtile[:, k, :],
                            ztile[:, k, mt * 512:(mt + 1) * 512],
                            start=(k == 0),
                            stop=(k == NB - 1),
                        )
                    otile = opool.tile([128, 512], mybir.dt.float32)
                    nc.vector.tensor_copy(otile[:], acc[:])
                    nc.sync.dma_start(
                        out=out_ext[i * 128:(i + 1) * 128, mt * 512:(mt + 1) * 512],
                        in_=otile[:],
                    )
    nc.finalize()
    _CACHE["dense"] = nc
    return nc


def _softmax(x, axis):
    x = x - x.max(axis=axis, keepdims=True)
    e = np.exp(x)
    return e / e.sum(axis=axis, keepdims=True)


def _epilogue(E, Zi, weights, reg):
    """E: (q, L, M) fp32; pl + reg."""
    mx = E.max(axis=0)
    lge = mx + np.log(np.sum(np.exp(E - mx[None]), axis=0))
    Ec = np.take_along_axis(E, Zi[None], axis=0)[0]
    pl = -float(np.sum(weights * np.sum(Ec - lge, axis=0)))
    return np.float32(pl + reg)


def _reg_exact(A, Vaa):
    """L2 reg from the exact per-q Atq (cheap on host, ~1.4 M elements)."""
    diag = Vaa[:, np.arange(Q_AA), np.arange(Q_AA)]
    Atq = np.einsum("hij,hq->qij", A, diag).astype(np.float32)
    Atq[:, np.arange(L), np.arange(L)] = 0.0
    return Atq, LAMBDA * float(np.sum(Atq.astype(np.float64) ** 2))


def _z_maps(Zi):
    Zf = Zi.astype(np.float16)                             # values 0..20 exact
    zs = []
    for c in range(N_CORES):
        zc = Zf[:, c * M_LOC:(c + 1) * M_LOC]              # (256, 1024)
        zs.append(np.ascontiguousarray(
            zc.reshape(2, 128, M_LOC).transpose(1, 0, 2)
        ))                                                 # (128, 2, 1024)
    return zs


def _fast_path(Atot, reg, Zi, weights):
    """Shared-Atot fp8 20-plane device path + host plane-21 reconstruction."""
    Atot8 = Atot.astype(_FP8_DT)
    at_np = np.ascontiguousarray(
        Atot8.T.reshape(2, 128, L).transpose(1, 0, 2)
    )                                                      # (128, 2, 256) fp8
    in_maps = [{"at": at_np, "z": z} for z in _z_maps(Zi)]

    nc = _build_fast_graph()
    res = run_bass_kernel_spmd(nc, in_maps, list(range(N_CORES)))
    E = np.empty((Q_AA, L, M_TOT), np.float32)
    E[:N_PLANES] = np.concatenate(
        [
            np.asarray(res.results[c]["e"])
            .astype(np.float32)                            # (q, 128, 2, m)
            .transpose(0, 2, 1, 3)                         # (q, rb, 128, m)
            .reshape(N_PLANES, L, M_LOC)
            for c in range(N_CORES)
        ],
        axis=2,
    ) * np.float32(1.0 / QSCALE)                           # dequantize
    # sum_q E_q == rowsum(fp8(Atot)) since the one-hot planes partition (j,m)
    rowsum = Atot8.astype(np.float32).sum(axis=1)          # (L,)
    E[N_PLANES] = rowsum[:, None] - E[:N_PLANES].sum(axis=0)
    return _epilogue(E, Zi, weights, reg)


def _perq_path(Atq, reg, Zi, weights):
    """Per-q 21-plane device path (diag of Vaa not uniform)."""
    AtT = Atq.transpose(0, 2, 1)                           # (q, j, r)
    at_np = np.ascontiguousarray(
        AtT.reshape(Q_AA, 2, 128, L).transpose(0, 2, 1, 3)
    ).astype(np.float16)                                   # (q, 128, 2, L)
    in_maps = [{"at": at_np, "z": z} for z in _z_maps(Zi)]

    nc = _build_perq_graph()
    res = run_bass_kernel_spmd(nc, in_maps, list(range(N_CORES)))
    E = np.concatenate(
        [
            np.asarray(res.results[c]["e"])
            .astype(np.float32)
            .transpose(0, 2, 1, 3)
            .reshape(Q_AA, L, M_LOC)
            for c in range(N_CORES)
        ],
        axis=2,
    )                                                      # (q, L, M)
    return _epilogue(E, Zi, weights, reg)


def _fast_path_host(Atq, reg, Zi, weights):
    """Numpy fallback of the per-q formulation (if device paths fail)."""
    E = np.empty((Q_AA, L, M_TOT), np.float32)
    for q in range(Q_AA):
        E[q] = Atq[q] @ (Zi == q).astype(np.float32)
    return _epilogue(E, Zi, weights, reg)


def _dense_path(A, Vaa, Zi, weights):
    J = (A.reshape(H, L * L).T @ Vaa.reshape(H, Q_AA * Q_AA)).reshape(
        L, L, Q_AA, Q_AA
    )
    J[np.arange(L), np.arange(L)] = 0.0
    reg = LAMBDA * float(np.sum(J.astype(np.float64) ** 2))

    Jmat = np.ascontiguousarray(J.transpose(0, 2, 1, 3).reshape(F, F))
    JT4 = np.ascontiguousarray(Jmat.T).reshape(NB, 128, NB, 128)
    jt_np = np.ascontiguousarray(JT4.transpose(2, 1, 0, 3)).astype(
        ml_dtypes.bfloat16
    )

    colidx = np.arange(L)[:, None] * Q_AA + Zi             # (L, M)
    in_maps = []
    for c in range(N_CORES):
        ci = colidx[:, c * M_LOC:(c + 1) * M_LOC]
        zfull = np.zeros((F, M_LOC), np.float32)
        zfull[ci, np.arange(M_LOC)[None, :]] = 1.0
        zoh_np = np.ascontiguousarray(
            zfull.reshape(NB, 128, M_LOC).transpose(1, 0, 2)
        ).astype(ml_dtypes.bfloat16)
        in_maps.append({"jt": jt_np, "zoh": zoh_np})

    try:
        nc = _build_dense_graph()
        res = run_bass_kernel_spmd(nc, in_maps, list(range(N_CORES)))
        E = np.concatenate(
            [np.asarray(res.results[c]["out"]).astype(np.float32)
             for c in range(N_CORES)], axis=1
        )
    except Exception:
        shards = []
        for c in range(N_CORES):
            ci = colidx[:, c * M_LOC:(c + 1) * M_LOC]
            zfull = np.zeros((F, M_LOC), np.float32)
            zfull[ci, np.arange(M_LOC)[None, :]] = 1.0
            shards.append(Jmat @ zfull)
        E = np.concatenate(shards, axis=1)

    E3 = np.ascontiguousarray(E.reshape(L, Q_AA, M_TOT).transpose(1, 0, 2))
    return _epilogue(E3, Zi, weights, reg)


def kernel(reps_matrix, Q, K, V_metric, Z, weights):
    reps_matrix = np.asarray(reps_matrix, np.float32)
    Q = np.asarray(Q, np.float32)
    K = np.asarray(K, np.float32)
    V_metric = np.asarray(V_metric, np.float32)
    Zi = np.asarray(Z).astype(np.int64)
    weights = np.asarray(weights, np.float32)

    # --- host prologue: attention map + RBF kernel ---
    scores = np.einsum("hid,hjd->hij", Q, K) / np.sqrt(np.float32(DK))
    probs = _softmax(scores, axis=-1)
    A = 0.5 * (probs + probs.transpose(0, 2, 1))           # (H, L, L)

    V1 = np.einsum("qd,hdv->hqv", reps_matrix, V_metric)   # (H, q, dv)
    gamma = 1.0 / V1.shape[1]
    sq = np.sum(V1 * V1, axis=-1)
    D2 = sq[:, :, None] + sq[:, None, :] - 2.0 * np.einsum("hqv,hav->hqa", V1, V1)
    Vaa = np.exp(-gamma * np.maximum(D2, 0.0))             # (H, q, q)

    # dropped-residual bound for the fast paths: |E_res| <= max_offdiag(Vaa)
    # * max row-sum of sum_h A[h]
    offmax = float((Vaa * (1.0 - np.eye(Q_AA, dtype=np.float32))[None]).max())
    rowsum = float(np.abs(A).sum(axis=0).sum(axis=1).max())
    diagdev = float(
        np.abs(Vaa[:, np.arange(Q_AA), np.arange(Q_AA)] - 1.0).max()
    )
    if offmax * rowsum < 1e-7:
        Atq, reg = _reg_exact(A, Vaa)
        try:
            Atot = A.sum(axis=0).astype(np.float32)
            Atot[np.arange(L), np.arange(L)] = 0.0
            rs_max = float(Atot.astype(_FP8_DT).astype(np.float32).sum(1).max())
            if diagdev * rowsum < 1e-2 and rs_max * QSCALE < 250.0:
                # Vaa ~ I enough to share one Atot, and E fits uint8 range
                return _fast_path(Atot, reg, Zi, weights)
            return _perq_path(Atq, reg, Zi, weights)
        except Exception:
            return _fast_path_host(Atq, reg, Zi, weights)
    return _dense_path(A, Vaa, Zi, weights)


# revision 7
# speedup vs baseline: 1.3026x; 1.0078x over previous
"""AttentionDCA pseudo-likelihood loss on 8 Trainium2 NeuronCores — fast path.

Mathematical structure exploited: with this problem's data distribution the
RBF kernel Vaa[h] = exp(-gamma*D2) is numerically an identity matrix
(off-diagonal entries < 1e-20, since D2 ~ 4096 and gamma = 1/21; diagonal
within 5e-5 of 1).  Writing Vaa[h] = I + R_h, the coupling tensor collapses:

    J[r,j,q,a] = Atot[r,j] * delta(q,a) + (residual, bounded by max R)
    Atot       = sum_h A[h],  diagonal (r==j) zeroed

so the energy tensor is 21 small matmuls instead of one 5376x5376 GEMM:

    E[q,r,m] = sum_j Atot[r,j] * [Z[j,m] == q]

and because sum_q [Z[j,m]==q] == 1, the planes satisfy
sum_q E[q,r,m] = rowsum(Atot)[r]: the device only computes 20 of the 21
planes and the host reconstructs the last one by subtraction.

Per core (M sharded 1024-per-core): 20 x (256x256)@(256x1024) fp16 matmuls
(~2.7 GFLOP, ~37 us of PE) with the one-hot RHS built on-device from Z by
DVE is_equal compares (4x 16-bit mode).  PSUM is drained fp32->fp16 split
5:3 across ACT:DVE; E returns in fp16 (10.5 MB); host does the cheap
logsumexp / gather / weighted-sum epilogue and the exact per-q L2 reg.

Tiered fallbacks, guarded by exact bounds on the dropped terms:
  1. shared-Atot 20-plane path   (needs off-diag Vaa ~ 0 AND uniform diag)
  2. per-q Atq 21-plane path     (needs off-diag Vaa ~ 0 only)
  3. dense J-matmul device path  (always valid; numpy as a last resort)
"""

import sys
import numpy as np

for p in ("/opt/trn_rl_repo", "/root/.axon_site/_ro/trn_rl_repo"):
    if p not in sys.path:
        sys.path.insert(0, p)

import ml_dtypes

import concourse.bass as bass
from concourse import bacc, mybir, tile
from concourse.bass_utils import run_bass_kernel_spmd

Q_AA = 21
H = 32
L = 256
DK = 32
M_TOT = 8192
N_CORES = 8
M_LOC = M_TOT // N_CORES          # 1024
F = L * Q_AA                      # 5376 flattened (pos, aa) dim
NB = F // 128                     # 42 blocks of 128
LAMBDA = 1e-3
N_PLANES = Q_AA - 1               # device computes 20, host rebuilds the 21st

_CACHE = {}


# byte pattern of fp8(1.0) for the one-hot synthesis trick below
_FP8_DT = np.dtype(mybir.dt.np(mybir.dt.float8e4))
_FP8_ONE_BYTE = int(np.array(1.0, _FP8_DT).view(np.uint8))
# uint8 output quantization scale; E in [0, rowsum_max], needs
# rowsum_max * QSCALE < 255 (guarded in kernel(); rowsum_max ~ 34 here)
QSCALE = 6.0


# --------------------------------------------------------------------------
# fast graph (shared Atot, fp8 DoubleRow): E[q,r,m] = sum_j Atot[r,j] *
# [Z[j,m]==q], planes q = 0..19 only; plane 20 reconstructed on host.
#
# The one-hot is built as uint16 words (z==q)*0x38: the LOW byte of each
# word is fp8(1.0)/fp8(0.0), so a stride-2 fp8 bitcast view of the uint16
# tile IS the fp8 one-hot plane — produced by a single 4x-mode DVE op.
# With both operands fp8, MatmulPerfMode.DoubleRow contracts K=256 in one
# instruction at 0.5 cycles/row (PE ~10 us instead of ~39).
# --------------------------------------------------------------------------
def _build_fast_graph():
    if "fast" in _CACHE:
        return _CACHE["fast"]
    nc = bacc.Bacc(None, target_bir_lowering=False)
    # lhsT: at[k(128), kb, r] fp8 = Atot.T[kb*128+k, r] (one 64 KB tile
    # shared by every plane; DoubleRow reads it as weights[p, two=kb, f=r])
    at_ext = nc.declare_dram_parameter(
        "at", [128, 2, L], mybir.dt.float8e4, isOutput=False
    )
    # z[k(128), kb, m] fp16 = Z[kb*128+k, m] as float
    z_ext = nc.declare_dram_parameter(
        "z", [128, 2, M_LOC], mybir.dt.float16, isOutput=False
    )
    # DRAM layout matches the SBUF tile's (partition-major) element order:
    # e[q, k, rb, m] with output row r = rb*128 + k
    # output quantized uint8: E*QSCALE rounded-to-nearest-even (verified
    # device semantics, saturating). Halves the dominant output DMA bytes.
    e_ext = nc.declare_dram_parameter(
        "e", [N_PLANES, 128, 2, M_LOC], mybir.dt.uint8, isOutput=True
    )

    with tile.TileContext(nc) as tc:
        with (
            tc.tile_pool(name="const", bufs=1) as cpool,
            tc.tile_pool(name="out", bufs=6) as opool,
            tc.tile_pool(name="psum", bufs=4, space=bass.MemorySpace.PSUM) as ppool,
        ):
            zt = cpool.tile([128, 2, M_LOC], mybir.dt.float16)
            at = cpool.tile([128, 2, L], mybir.dt.float8e4)
            # all one-hot planes stay resident as uint16; synthesis runs a
            # few planes ahead of the matmul stream on DVE
            zq = cpool.tile([128, N_PLANES, 2, M_LOC], mybir.dt.uint16)

            nc.sync.dma_start(out=zt[:], in_=z_ext[:])
            nc.sync.dma_start(out=at[:], in_=at_ext[:])

            # two planes synthesized on the otherwise-idle Pool engine to
            # shave the DVE stream (Pool is ~5x slower per op but has slack)
            pool_onehots = (9, 15)
            issued = set()

            def _cmp(q):
                # (z == q) -> 1/0 uint16, * 0x38 -> low byte = fp8(1.0)
                if q in issued or q >= N_PLANES:
                    return
                issued.add(q)
                eng = nc.gpsimd if q in pool_onehots else nc.vector
                eng.tensor_scalar(
                    zq[:, q], zt[:], float(q), _FP8_ONE_BYTE,
                    mybir.AluOpType.is_equal, mybir.AluOpType.mult,
                )

            for q in sorted(pool_onehots):
                _cmp(q)
            for q in range(3):
                _cmp(q)

            ci = 0
            for q in range(N_PLANES):
                for fq in range(q + 1, q + 4):
                    _cmp(fq)
                otile = opool.tile([128, 2, M_LOC], mybir.dt.uint8)
                # stride-2 fp8 view of the uint16 words = the one-hot plane
                zq8 = zq[:, q].bitcast(mybir.dt.float8e4)  # [128, 2, 2048]
                for rb in range(2):
                    acc = ppool.tile([128, 1024], mybir.dt.float32)
                    for mt in range(M_LOC // 512):
                        rhs = zq8[:, :, mt * 1024:(mt + 1) * 1024:2]
                        nc.tensor.matmul(
                            acc[:, mt * 512:(mt + 1) * 512],
                            at[:, :, rb * 128:(rb + 1) * 128],
                            rhs,
                            start=True,
                            stop=True,
                            perf_mode=mybir.MatmulPerfMode.DoubleRow,
                        )
                    # one merged [128,1024] PSUM->SBUF drain per rb that
                    # also quantizes E*QSCALE -> uint8, split 2:1 ACT:DVE
                    # (DVE also synthesizes the one-hots; gpsimd cannot
                    # read PSUM here)
                    if ci % 8 in (2, 5, 7):
                        nc.vector.tensor_scalar(
                            otile[:, rb], acc[:], QSCALE, None,
                            mybir.AluOpType.mult,
                        )
                    else:
                        nc.scalar.activation(
                            otile[:, rb], acc[:],
                            mybir.ActivationFunctionType.Copy, 0.0, QSCALE,
                        )
                    # per-rb output DMAs pipeline the writeback finer;
                    # alternate between the SP/HWDGE queue and the Pool/SWDGE
                    # queue (two parallel DMA issue paths — HWDGE generation
                    # at 625 ns/DMA otherwise congests the writeback tail)
                    if ci % 2 == 0:
                        nc.gpsimd.dma_start(out=e_ext[q, :, rb], in_=otile[:, rb])
                    else:
                        nc.sync.dma_start(out=e_ext[q, :, rb], in_=otile[:, rb])
                    ci += 1
    nc.finalize()   # Bacc.compile(): sync legalization + reg allocation
    _CACHE["fast"] = nc
    return nc


# --------------------------------------------------------------------------
# per-q graph: E[q,r,m] = sum_j Atq[q][r,j] * [Z[j,m]==q], all 21 planes
# (used when the Vaa diagonal is not uniform enough for the shared-Atot
# shortcut)
# --------------------------------------------------------------------------
def _build_perq_graph():
    if "perq" in _CACHE:
        return _CACHE["perq"]
    nc = bacc.Bacc(None, target_bir_lowering=False)
    at_ext = nc.declare_dram_parameter(
        "at", [Q_AA, 128, 2, L], mybir.dt.float16, isOutput=False
    )
    z_ext = nc.declare_dram_parameter(
        "z", [128, 2, M_LOC], mybir.dt.float16, isOutput=False
    )
    e_ext = nc.declare_dram_parameter(
        "e", [Q_AA, 128, 2, M_LOC], mybir.dt.float16, isOutput=True
    )

    with tile.TileContext(nc) as tc:
        with (
            tc.tile_pool(name="const", bufs=1) as cpool,
            tc.tile_pool(name="out", bufs=6) as opool,
            tc.tile_pool(name="psum", bufs=8, space=bass.MemorySpace.PSUM) as ppool,
        ):
            zt = cpool.tile([128, 2, M_LOC], mybir.dt.float16)
            at = cpool.tile([128, Q_AA, 2, L], mybir.dt.float16)
            zq = cpool.tile([128, Q_AA, 2, M_LOC], mybir.dt.float16)

            nc.sync.dma_start(out=zt[:], in_=z_ext[:])
            for q in range(Q_AA):
                nc.sync.dma_start(out=at[:, q], in_=at_ext[q])

            def _cmp(q):
                nc.vector.tensor_scalar(
                    zq[:, q], zt[:], float(q), None, mybir.AluOpType.is_equal
                )

            lookahead = 4
            for q in range(min(lookahead, Q_AA)):
                _cmp(q)

            ci = 0
            for q in range(Q_AA):
                if q + lookahead < Q_AA:
                    _cmp(q + lookahead)
                otile = opool.tile([128, 2, M_LOC], mybir.dt.float16)
                for rb in range(2):
                    for mt in range(M_LOC // 512):
                        acc = ppool.tile([128, 512], mybir.dt.float32)
                        for kb in range(2):
                            nc.tensor.matmul(
                                acc[:],
                                at[:, q, kb, rb * 128:(rb + 1) * 128],
                                zq[:, q, kb, mt * 512:(mt + 1) * 512],
                                start=(kb == 0),
                                stop=(kb == 1),
                            )
                        if ci % 3 == 2:
                            nc.vector.tensor_copy(
                                otile[:, rb, mt * 512:(mt + 1) * 512], acc[:]
                            )
                        else:
                            nc.scalar.copy(
                                otile[:, rb, mt * 512:(mt + 1) * 512], acc[:]
                            )
                        ci += 1
                    nc.sync.dma_start(out=e_ext[q, :, rb], in_=otile[:, rb])
    nc.finalize()
    _CACHE["perq"] = nc
    return nc


# --------------------------------------------------------------------------
# dense fallback graph: E = Jmat @ Zoh  (always valid)
# --------------------------------------------------------------------------
def _build_dense_graph():
    if "dense" in _CACHE:
        return _CACHE["dense"]
    nc = bacc.Bacc(None, target_bir_lowering=False)
    jt_ext = nc.declare_dram_parameter(
        "jt", [NB, 128, NB, 128], mybir.dt.bfloat16, isOutput=False
    )
    zoh_ext = nc.declare_dram_parameter(
        "zoh", [128, NB, M_LOC], mybir.dt.bfloat16, isOutput=False
    )
    out_ext = nc.declare_dram_parameter(
        "out", [F, M_LOC], mybir.dt.float32, isOutput=True
    )

    with tile.TileContext(nc) as tc:
        with (
            tc.tile_pool(name="zpool", bufs=1) as zpool,
            tc.tile_pool(name="jpool", bufs=3) as jpool,
            tc.tile_pool(name="opool", bufs=4) as opool,
            tc.tile_pool(name="psum", bufs=4, space=bass.MemorySpace.PSUM) as ppool,
        ):
            ztile = zpool.tile([128, NB, M_LOC], mybir.dt.bfloat16)
            nc.sync.dma_start(out=ztile[:], in_=zoh_ext[:])

            for i in range(NB):
                jtile = jpool.tile([128, NB, 128], mybir.dt.bfloat16)
                nc.sync.dma_start(out=jtile[:], in_=jt_ext[i])
                for mt in range(M_LOC // 512):
                    acc = ppool.tile([128, 512], mybir.dt.float32)
                    for k in range(NB):
                        nc.tensor.matmul(
                            acc[:],
                            jtile[:, k, :],
                            ztile[:, k, mt * 512:(mt + 1) * 512],
                            start=(k == 0),
                            stop=(k == NB - 1),
                        )
                    otile = opool.tile([128, 512], mybir.dt.float32)
                    nc.vector.tensor_copy(otile[:], acc[:])
                    nc.sync.dma_start(
                        out=out_ext[i * 128:(i + 1) * 128, mt * 512:(mt + 1) * 512],
                        in_=otile[:],
                    )
    nc.finalize()
    _CACHE["dense"] = nc
    return nc


def _softmax(x, axis):
    x = x - x.max(axis=axis, keepdims=True)
    e = np.exp(x)
    return e / e.sum(axis=axis, keepdims=True)


def _epilogue(E, Zi, weights, reg):
    """E: (q, L, M) fp32; pl + reg."""
    mx = E.max(axis=0)
    lge = mx + np.log(np.sum(np.exp(E - mx[None]), axis=0))
    Ec = np.take_along_axis(E, Zi[None], axis=0)[0]
    pl = -float(np.sum(weights * np.sum(Ec - lge, axis=0)))
    return np.float32(pl + reg)


def _reg_exact(A, Vaa):
    """L2 reg from the exact per-q Atq (cheap on host, ~1.4 M elements)."""
    diag = Vaa[:, np.arange(Q_AA), np.arange(Q_AA)]
    Atq = np.einsum("hij,hq->qij", A, diag).astype(np.float32)
    Atq[:, np.arange(L), np.arange(L)] = 0.0
    return Atq, LAMBDA * float(np.sum(Atq.astype(np.float64) ** 2))


def _z_maps(Zi):
    Zf = Zi.astype(np.float16)                             # values 0..20 exact
    zs = []
    for c in range(N_CORES):
        zc = Zf[:, c * M_LOC:(c + 1) * M_LOC]              # (256, 1024)
        zs.append(np.ascontiguousarray(
            zc.reshape(2, 128, M_LOC).transpose(1, 0, 2)
        ))                                                 # (128, 2, 1024)
    return zs


def _fast_path(Atot, reg, Zi, weights):
    """Shared-Atot fp8 20-plane device path + host plane-21 reconstruction."""
    Atot8 = Atot.astype(_FP8_DT)
    at_np = np.ascontiguousarray(
        Atot8.T.reshape(2, 128, L).transpose(1, 0, 2)
    )                                                      # (128, 2, 256) fp8
    in_maps = [{"at": at_np, "z": z} for z in _z_maps(Zi)]

    nc = _build_fast_graph()
    res = run_bass_kernel_spmd(nc, in_maps, list(range(N_CORES)))
    E = np.empty((Q_AA, L, M_TOT), np.float32)
    E[:N_PLANES] = np.concatenate(
        [
            np.asarray(res.results[c]["e"])
            .astype(np.float32)                            # (q, 128, 2, m)
            .transpose(0, 2, 1, 3)                         # (q, rb, 128, m)
            .reshape(N_PLANES, L, M_LOC)
            for c in range(N_CORES)
        ],
        axis=2,
    ) * np.float32(1.0 / QSCALE)                           # dequantize
    # sum_q E_q == rowsum(fp8(Atot)) since the one-hot planes partition (j,m)
    rowsum = Atot8.astype(np.float32).sum(axis=1)          # (L,)
    E[N_PLANES] = rowsum[:, None] - E[:N_PLANES].sum(axis=0)
    return _epilogue(E, Zi, weights, reg)


def _perq_path(Atq, reg, Zi, weights):
    """Per-q 21-plane device path (diag of Vaa not uniform)."""
    AtT = Atq.transpose(0, 2, 1)                           # (q, j, r)
    at_np = np.ascontiguousarray(
        AtT.reshape(Q_AA, 2, 128, L).transpose(0, 2, 1, 3)
    ).astype(np.float16)                                   # (q, 128, 2, L)
    in_maps = [{"at": at_np, "z": z} for z in _z_maps(Zi)]

    nc = _build_perq_graph()
    res = run_bass_kernel_spmd(nc, in_maps, list(range(N_CORES)))
    E = np.concatenate(
        [
            np.asarray(res.results[c]["e"])
            .astype(np.float32)
            .transpose(0, 2, 1, 3)
            .reshape(Q_AA, L, M_LOC)
            for c in range(N_CORES)
        ],
        axis=2,
    )                                                      # (q, L, M)
    return _epilogue(E, Zi, weights, reg)


def _fast_path_host(Atq, reg, Zi, weights):
    """Numpy fallback of the per-q formulation (if device paths fail)."""
    E = np.empty((Q_AA, L, M_TOT), np.float32)
    for q in range(Q_AA):
        E[q] = Atq[q] @ (Zi == q).astype(np.float32)
    return _epilogue(E, Zi, weights, reg)


def _dense_path(A, Vaa, Zi, weights):
    J = (A.reshape(H, L * L).T @ Vaa.reshape(H, Q_AA * Q_AA)).reshape(
        L, L, Q_AA, Q_AA
    )
    J[np.arange(L), np.arange(L)] = 0.0
    reg = LAMBDA * float(np.sum(J.astype(np.float64) ** 2))

    Jmat = np.ascontiguousarray(J.transpose(0, 2, 1, 3).reshape(F, F))
    JT4 = np.ascontiguousarray(Jmat.T).reshape(NB, 128, NB, 128)
    jt_np = np.ascontiguousarray(JT4.transpose(2, 1, 0, 3)).astype(
        ml_dtypes.bfloat16
    )

    colidx = np.arange(L)[:, None] * Q_AA + Zi             # (L, M)
    in_maps = []
    for c in range(N_CORES):
        ci = colidx[:, c * M_LOC:(c + 1) * M_LOC]
        zfull = np.zeros((F, M_LOC), np.float32)
        zfull[ci, np.arange(M_LOC)[None, :]] = 1.0
        zoh_np = np.ascontiguousarray(
            zfull.reshape(NB, 128, M_LOC).transpose(1, 0, 2)
        ).astype(ml_dtypes.bfloat16)
        in_maps.append({"jt": jt_np, "zoh": zoh_np})

    try:
        nc = _build_dense_graph()
        res = run_bass_kernel_spmd(nc, in_maps, list(range(N_CORES)))
        E = np.concatenate(
            [np.asarray(res.results[c]["out"]).astype(np.float32)
             for c in range(N_CORES)], axis=1
        )
    except Exception:
        shards = []
        for c in range(N_CORES):
            ci = colidx[:, c * M_LOC:(c + 1) * M_LOC]
            zfull = np.zeros((F, M_LOC), np.float32)
            zfull[ci, np.arange(M_LOC)[None, :]] = 1.0
            shards.append(Jmat @ zfull)
        E = np.concatenate(shards, axis=1)

    E3 = np.ascontiguousarray(E.reshape(L, Q_AA, M_TOT).transpose(1, 0, 2))
    return _epilogue(E3, Zi, weights, reg)


def kernel(reps_matrix, Q, K, V_metric, Z, weights):
    reps_matrix = np.asarray(reps_matrix, np.float32)
    Q = np.asarray(Q, np.float32)
    K = np.asarray(K, np.float32)
    V_metric = np.asarray(V_metric, np.float32)
    Zi = np.asarray(Z).astype(np.int64)
    weights = np.asarray(weights, np.float32)

    # --- host prologue: attention map + RBF kernel ---
    scores = np.einsum("hid,hjd->hij", Q, K) / np.sqrt(np.float32(DK))
    probs = _softmax(scores, axis=-1)
    A = 0.5 * (probs + probs.transpose(0, 2, 1))           # (H, L, L)

    V1 = np.einsum("qd,hdv->hqv", reps_matrix, V_metric)   # (H, q, dv)
    gamma = 1.0 / V1.shape[1]
    sq = np.sum(V1 * V1, axis=-1)
    D2 = sq[:, :, None] + sq[:, None, :] - 2.0 * np.einsum("hqv,hav->hqa", V1, V1)
    Vaa = np.exp(-gamma * np.maximum(D2, 0.0))             # (H, q, q)

    # dropped-residual bound for the fast paths: |E_res| <= max_offdiag(Vaa)
    # * max row-sum of sum_h A[h]
    offmax = float((Vaa * (1.0 - np.eye(Q_AA, dtype=np.float32))[None]).max())
    rowsum = float(np.abs(A).sum(axis=0).sum(axis=1).max())
    diagdev = float(
        np.abs(Vaa[:, np.arange(Q_AA), np.arange(Q_AA)] - 1.0).max()
    )
    if offmax * rowsum < 1e-7:
        Atq, reg = _reg_exact(A, Vaa)
        try:
            Atot = A.sum(axis=0).astype(np.float32)
            Atot[np.arange(L), np.arange(L)] = 0.0
            rs_max = float(Atot.astype(_FP8_DT).astype(np.float32).sum(1).max())
            if diagdev * rowsum < 1e-2 and rs_max * QSCALE < 250.0:
                # Vaa ~ I enough to share one Atot, and E fits uint8 range
                return _fast_path(Atot, reg, Zi, weights)
            return _perq_path(Atq, reg, Zi, weights)
        except Exception:
            return _fast_path_host(Atq, reg, Zi, weights)
    return _dense_path(A, Vaa, Zi, weights)


# revision 8
# speedup vs baseline: 1.3045x; 1.0014x over previous
"""AttentionDCA pseudo-likelihood loss on 8 Trainium2 NeuronCores — fast path.

Mathematical structure exploited: with this problem's data distribution the
RBF kernel Vaa[h] = exp(-gamma*D2) is numerically an identity matrix
(off-diagonal entries < 1e-20, since D2 ~ 4096 and gamma = 1/21; diagonal
within 5e-5 of 1).  Writing Vaa[h] = I + R_h, the coupling tensor collapses:

    J[r,j,q,a] = Atot[r,j] * delta(q,a) + (residual, bounded by max R)
    Atot       = sum_h A[h],  diagonal (r==j) zeroed

so the energy tensor is 21 small matmuls instead of one 5376x5376 GEMM:

    E[q,r,m] = sum_j Atot[r,j] * [Z[j,m] == q]

and because sum_q [Z[j,m]==q] == 1, the planes satisfy
sum_q E[q,r,m] = rowsum(Atot)[r]: the device only computes 20 of the 21
planes and the host reconstructs the last one by subtraction.

Per core (M sharded 1024-per-core): 20 x (256x256)@(256x1024) fp16 matmuls
(~2.7 GFLOP, ~37 us of PE) with the one-hot RHS built on-device from Z by
DVE is_equal compares (4x 16-bit mode).  PSUM is drained fp32->fp16 split
5:3 across ACT:DVE; E returns in fp16 (10.5 MB); host does the cheap
logsumexp / gather / weighted-sum epilogue and the exact per-q L2 reg.

Tiered fallbacks, guarded by exact bounds on the dropped terms:
  1. shared-Atot 20-plane path   (needs off-diag Vaa ~ 0 AND uniform diag)
  2. per-q Atq 21-plane path     (needs off-diag Vaa ~ 0 only)
  3. dense J-matmul device path  (always valid; numpy as a last resort)
"""

import sys
import numpy as np

for p in ("/opt/trn_rl_repo", "/root/.axon_site/_ro/trn_rl_repo"):
    if p not in sys.path:
        sys.path.insert(0, p)

import ml_dtypes

import concourse.bass as bass
from concourse import bacc, mybir, tile
from concourse.bass_utils import run_bass_kernel_spmd

Q_AA = 21
H = 32
L = 256
DK = 32
M_TOT = 8192
N_CORES = 8
M_LOC = M_TOT // N_CORES          # 1024
F = L * Q_AA                      # 5376 flattened (pos, aa) dim
NB = F // 128                     # 42 blocks of 128
LAMBDA = 1e-3
N_PLANES = Q_AA - 1               # device computes 20, host rebuilds the 21st

_CACHE = {}


# byte pattern of fp8(1.0) for the one-hot synthesis trick below
_FP8_DT = np.dtype(mybir.dt.np(mybir.dt.float8e4))
_FP8_ONE_BYTE = int(np.array(1.0, _FP8_DT).view(np.uint8))
# uint8 output quantization scale; E in [0, rowsum_max], needs
# rowsum_max * QSCALE < 255 (guarded in kernel(); rowsum_max ~ 34 here)
QSCALE = 6.0


# --------------------------------------------------------------------------
# fast graph (shared Atot, fp8 DoubleRow): E[q,r,m] = sum_j Atot[r,j] *
# [Z[j,m]==q], planes q = 0..19 only; plane 20 reconstructed on host.
#
# The one-hot is built as uint16 words (z==q)*0x38: the LOW byte of each
# word is fp8(1.0)/fp8(0.0), so a stride-2 fp8 bitcast view of the uint16
# tile IS the fp8 one-hot plane — produced by a single 4x-mode DVE op.
# With both operands fp8, MatmulPerfMode.DoubleRow contracts K=256 in one
# instruction at 0.5 cycles/row (PE ~10 us instead of ~39).
# --------------------------------------------------------------------------
def _build_fast_graph():
    if "fast" in _CACHE:
        return _CACHE["fast"]
    nc = bacc.Bacc(None, target_bir_lowering=False)
    # lhsT: at[k(128), kb, r] fp8 = Atot.T[kb*128+k, r] (one 64 KB tile
    # shared by every plane; DoubleRow reads it as weights[p, two=kb, f=r])
    at_ext = nc.declare_dram_parameter(
        "at", [128, 2, L], mybir.dt.float8e4, isOutput=False
    )
    # z[k(128), kb, m] fp16 = Z[kb*128+k, m] as float
    z_ext = nc.declare_dram_parameter(
        "z", [128, 2, M_LOC], mybir.dt.float16, isOutput=False
    )
    # DRAM layout matches the SBUF tile's (partition-major) element order:
    # e[q, k, rb, m] with output row r = rb*128 + k
    # output quantized uint8: E*QSCALE rounded-to-nearest-even (verified
    # device semantics, saturating). Halves the dominant output DMA bytes.
    e_ext = nc.declare_dram_parameter(
        "e", [N_PLANES, 128, 2, M_LOC], mybir.dt.uint8, isOutput=True
    )

    with tile.TileContext(nc) as tc:
        with (
            tc.tile_pool(name="const", bufs=1) as cpool,
            tc.tile_pool(name="out", bufs=6) as opool,
            tc.tile_pool(name="psum", bufs=4, space=bass.MemorySpace.PSUM) as ppool,
        ):
            zt = cpool.tile([128, 2, M_LOC], mybir.dt.float16)
            at = cpool.tile([128, 2, L], mybir.dt.float8e4)
            # all one-hot planes stay resident as uint16; synthesis runs a
            # few planes ahead of the matmul stream on DVE
            zq = cpool.tile([128, N_PLANES, 2, M_LOC], mybir.dt.uint16)

            nc.sync.dma_start(out=zt[:], in_=z_ext[:])
            nc.sync.dma_start(out=at[:], in_=at_ext[:])

            # two planes synthesized on the otherwise-idle Pool engine to
            # shave the DVE stream (Pool is ~5x slower per op but has slack)
            pool_onehots = (9, 15)
            issued = set()

            def _cmp(q):
                # (z == q) -> 1/0 uint16, * 0x38 -> low byte = fp8(1.0)
                if q in issued or q >= N_PLANES:
                    return
                issued.add(q)
                eng = nc.gpsimd if q in pool_onehots else nc.vector
                eng.tensor_scalar(
                    zq[:, q], zt[:], float(q), _FP8_ONE_BYTE,
                    mybir.AluOpType.is_equal, mybir.AluOpType.mult,
                )

            for q in sorted(pool_onehots):
                _cmp(q)
            lookahead = 5
            for q in range(lookahead):
                _cmp(q)

            ci = 0
            for q in range(N_PLANES):
                for fq in range(q + 1, q + 1 + lookahead):
                    _cmp(fq)
                otile = opool.tile([128, 2, M_LOC], mybir.dt.uint8)
                # stride-2 fp8 view of the uint16 words = the one-hot plane
                zq8 = zq[:, q].bitcast(mybir.dt.float8e4)  # [128, 2, 2048]
                for rb in range(2):
                    acc = ppool.tile([128, 1024], mybir.dt.float32)
                    for mt in range(M_LOC // 512):
                        rhs = zq8[:, :, mt * 1024:(mt + 1) * 1024:2]
                        nc.tensor.matmul(
                            acc[:, mt * 512:(mt + 1) * 512],
                            at[:, :, rb * 128:(rb + 1) * 128],
                            rhs,
                            start=True,
                            stop=True,
                            perf_mode=mybir.MatmulPerfMode.DoubleRow,
                        )
                    # one merged [128,1024] PSUM->SBUF drain per rb that
                    # also quantizes E*QSCALE -> uint8, split 2:1 ACT:DVE
                    # (DVE also synthesizes the one-hots; gpsimd cannot
                    # read PSUM here)
                    if ci % 8 in (2, 5, 7):
                        nc.vector.tensor_scalar(
                            otile[:, rb], acc[:], QSCALE, None,
                            mybir.AluOpType.mult,
                        )
                    else:
                        nc.scalar.activation(
                            otile[:, rb], acc[:],
                            mybir.ActivationFunctionType.Copy, 0.0, QSCALE,
                        )
                    # per-rb output DMAs pipeline the writeback finer;
                    # alternate between the SP/HWDGE queue and the Pool/SWDGE
                    # queue (two parallel DMA issue paths — HWDGE generation
                    # at 625 ns/DMA otherwise congests the writeback tail)
                    if ci % 2 == 0:
                        nc.gpsimd.dma_start(out=e_ext[q, :, rb], in_=otile[:, rb])
                    else:
                        nc.sync.dma_start(out=e_ext[q, :, rb], in_=otile[:, rb])
                    ci += 1
    nc.finalize()   # Bacc.compile(): sync legalization + reg allocation
    _CACHE["fast"] = nc
    return nc


# --------------------------------------------------------------------------
# per-q graph: E[q,r,m] = sum_j Atq[q][r,j] * [Z[j,m]==q], all 21 planes
# (used when the Vaa diagonal is not uniform enough for the shared-Atot
# shortcut)
# --------------------------------------------------------------------------
def _build_perq_graph():
    if "perq" in _CACHE:
        return _CACHE["perq"]
    nc = bacc.Bacc(None, target_bir_lowering=False)
    at_ext = nc.declare_dram_parameter(
        "at", [Q_AA, 128, 2, L], mybir.dt.float16, isOutput=False
    )
    z_ext = nc.declare_dram_parameter(
        "z", [128, 2, M_LOC], mybir.dt.float16, isOutput=False
    )
    e_ext = nc.declare_dram_parameter(
        "e", [Q_AA, 128, 2, M_LOC], mybir.dt.float16, isOutput=True
    )

    with tile.TileContext(nc) as tc:
        with (
            tc.tile_pool(name="const", bufs=1) as cpool,
            tc.tile_pool(name="out", bufs=6) as opool,
            tc.tile_pool(name="psum", bufs=8, space=bass.MemorySpace.PSUM) as ppool,
        ):
            zt = cpool.tile([128, 2, M_LOC], mybir.dt.float16)
            at = cpool.tile([128, Q_AA, 2, L], mybir.dt.float16)
            zq = cpool.tile([128, Q_AA, 2, M_LOC], mybir.dt.float16)

            nc.sync.dma_start(out=zt[:], in_=z_ext[:])
            for q in range(Q_AA):
                nc.sync.dma_start(out=at[:, q], in_=at_ext[q])

            def _cmp(q):
                nc.vector.tensor_scalar(
                    zq[:, q], zt[:], float(q), None, mybir.AluOpType.is_equal
                )

            lookahead = 4
            for q in range(min(lookahead, Q_AA)):
                _cmp(q)

            ci = 0
            for q in range(Q_AA):
                if q + lookahead < Q_AA:
                    _cmp(q + lookahead)
                otile = opool.tile([128, 2, M_LOC], mybir.dt.float16)
                for rb in range(2):
                    for mt in range(M_LOC // 512):
                        acc = ppool.tile([128, 512], mybir.dt.float32)
                        for kb in range(2):
                            nc.tensor.matmul(
                                acc[:],
                                at[:, q, kb, rb * 128:(rb + 1) * 128],
                                zq[:, q, kb, mt * 512:(mt + 1) * 512],
                                start=(kb == 0),
                                stop=(kb == 1),
                            )
                        if ci % 3 == 2:
                            nc.vector.tensor_copy(
                                otile[:, rb, mt * 512:(mt + 1) * 512], acc[:]
                            )
                        else:
                            nc.scalar.copy(
                                otile[:, rb, mt * 512:(mt + 1) * 512], acc[:]
                            )
                        ci += 1
                    nc.sync.dma_start(out=e_ext[q, :, rb], in_=otile[:, rb])
    nc.finalize()
    _CACHE["perq"] = nc
    return nc


# --------------------------------------------------------------------------
# dense fallback graph: E = Jmat @ Zoh  (always valid)
# --------------------------------------------------------------------------
def _build_dense_graph():
    if "dense" in _CACHE:
        return _CACHE["dense"]
    nc = bacc.Bacc(None, target_bir_lowering=False)
    jt_ext = nc.declare_dram_parameter(
        "jt", [NB, 128, NB, 128], mybir.dt.bfloat16, isOutput=False
    )
    zoh_ext = nc.declare_dram_parameter(
        "zoh", [128, NB, M_LOC], mybir.dt.bfloat16, isOutput=False
    )
    out_ext = nc.declare_dram_parameter(
        "out", [F, M_LOC], mybir.dt.float32, isOutput=True
    )

    with tile.TileContext(nc) as tc:
        with (
            tc.tile_pool(name="zpool", bufs=1) as zpool,
            tc.tile_pool(name="jpool", bufs=3) as jpool,
            tc.tile_pool(name="opool", bufs=4) as opool,
            tc.tile_pool(name="psum", bufs=4, space=bass.MemorySpace.PSUM) as ppool,
        ):
            ztile = zpool.tile([128, NB, M_LOC], mybir.dt.bfloat16)
            nc.sync.dma_start(out=ztile[:], in_=zoh_ext[:])

            for i in range(NB):
                jtile = jpool.tile([128, NB, 128], mybir.dt.bfloat16)
                nc.sync.dma_start(out=jtile[:], in_=jt_ext[i])
                for mt in range(M_LOC // 512):
                    acc = ppool.tile([128, 512], mybir.dt.float32)
                    for k in range(NB):
                        nc.tensor.matmul(
                            acc[:],
                            jtile[:, k, :],
                            ztile[:, k, mt * 512:(mt + 1) * 512],
                            start=(k == 0),
                            stop=(k == NB - 1),
                        )
                    otile = opool.tile([128, 512], mybir.dt.float32)
                    nc.vector.tensor_copy(otile[:], acc[:])
                    nc.sync.dma_start(
                        out=out_ext[i * 128:(i + 1) * 128, mt * 512:(mt + 1) * 512],
                        in_=otile[:],
                    )
    nc.finalize()
    _CACHE["dense"] = nc
    return nc


def _softmax(x, axis):
    x = x - x.max(axis=axis, keepdims=True)
    e = np.exp(x)
    return e / e.sum(axis=axis, keepdims=True)


def _epilogue(E, Zi, weights, reg):
    """E: (q, L, M) fp32; pl + reg."""
    mx = E.max(axis=0)
    lge = mx + np.log(np.sum(np.exp(E - mx[None]), axis=0))
    Ec = np.take_along_axis(E, Zi[None], axis=0)[0]
    pl = -float(np.sum(weights * np.sum(Ec - lge, axis=0)))
    return np.float32(pl + reg)


def _reg_exact(A, Vaa):
    """L2 reg from the exact per-q Atq (cheap on host, ~1.4 M elements)."""
    diag = Vaa[:, np.arange(Q_AA), np.arange(Q_AA)]
    Atq = np.einsum("hij,hq->qij", A, diag).astype(np.float32)
    Atq[:, np.arange(L), np.arange(L)] = 0.0
    return Atq, LAMBDA * float(np.sum(Atq.astype(np.float64) ** 2))


def _z_maps(Zi):
    Zf = Zi.astype(np.float16)                             # values 0..20 exact
    zs = []
    for c in range(N_CORES):
        zc = Zf[:, c * M_LOC:(c + 1) * M_LOC]              # (256, 1024)
        zs.append(np.ascontiguousarray(
            zc.reshape(2, 128, M_LOC).transpose(1, 0, 2)
        ))                                                 # (128, 2, 1024)
    return zs


def _fast_path(Atot, reg, Zi, weights):
    """Shared-Atot fp8 20-plane device path + host plane-21 reconstruction."""
    Atot8 = Atot.astype(_FP8_DT)
    at_np = np.ascontiguousarray(
        Atot8.T.reshape(2, 128, L).transpose(1, 0, 2)
    )                                                      # (128, 2, 256) fp8
    in_maps = [{"at": at_np, "z": z} for z in _z_maps(Zi)]

    nc = _build_fast_graph()
    res = run_bass_kernel_spmd(nc, in_maps, list(range(N_CORES)))
    E = np.empty((Q_AA, L, M_TOT), np.float32)
    E[:N_PLANES] = np.concatenate(
        [
            np.asarray(res.results[c]["e"])
            .astype(np.float32)                            # (q, 128, 2, m)
            .transpose(0, 2, 1, 3)                         # (q, rb, 128, m)
            .reshape(N_PLANES, L, M_LOC)
            for c in range(N_CORES)
        ],
        axis=2,
    ) * np.float32(1.0 / QSCALE)                           # dequantize
    # sum_q E_q == rowsum(fp8(Atot)) since the one-hot planes partition (j,m)
    rowsum = Atot8.astype(np.float32).sum(axis=1)          # (L,)
    E[N_PLANES] = rowsum[:, None] - E[:N_PLANES].sum(axis=0)
    return _epilogue(E, Zi, weights, reg)


def _perq_path(Atq, reg, Zi, weights):
    """Per-q 21-plane device path (diag of Vaa not uniform)."""
    AtT = Atq.transpose(0, 2, 1)                           # (q, j, r)
    at_np = np.ascontiguousarray(
        AtT.reshape(Q_AA, 2, 128, L).transpose(0, 2, 1, 3)
    ).astype(np.float16)                                   # (q, 128, 2, L)
    in_maps = [{"at": at_np, "z": z} for z in _z_maps(Zi)]

    nc = _build_perq_graph()
    res = run_bass_kernel_spmd(nc, in_maps, list(range(N_CORES)))
    E = np.concatenate(
        [
            np.asarray(res.results[c]["e"])
            .astype(np.float32)
            .transpose(0, 2, 1, 3)
            .reshape(Q_AA, L, M_LOC)
            for c in range(N_CORES)
        ],
        axis=2,
    )                                                      # (q, L, M)
    return _epilogue(E, Zi, weights, reg)


def _fast_path_host(Atq, reg, Zi, weights):
    """Numpy fallback of the per-q formulation (if device paths fail)."""
    E = np.empty((Q_AA, L, M_TOT), np.float32)
    for q in range(Q_AA):
        E[q] = Atq[q] @ (Zi == q).astype(np.float32)
    return _epilogue(E, Zi, weights, reg)


def _dense_path(A, Vaa, Zi, weights):
    J = (A.reshape(H, L * L).T @ Vaa.reshape(H, Q_AA * Q_AA)).reshape(
        L, L, Q_AA, Q_AA
    )
    J[np.arange(L), np.arange(L)] = 0.0
    reg = LAMBDA * float(np.sum(J.astype(np.float64) ** 2))

    Jmat = np.ascontiguousarray(J.transpose(0, 2, 1, 3).reshape(F, F))
    JT4 = np.ascontiguousarray(Jmat.T).reshape(NB, 128, NB, 128)
    jt_np = np.ascontiguousarray(JT4.transpose(2, 1, 0, 3)).astype(
        ml_dtypes.bfloat16
    )

    colidx = np.arange(L)[:, None] * Q_AA + Zi             # (L, M)
    in_maps = []
    for c in range(N_CORES):
        ci = colidx[:, c * M_LOC:(c + 1) * M_LOC]
        zfull = np.zeros((F, M_LOC), np.float32)
        zfull[ci, np.arange(M_LOC)[None, :]] = 1.0
        zoh_np = np.ascontiguousarray(
            zfull.reshape(NB, 128, M_LOC).transpose(1, 0, 2)
        ).astype(ml_dtypes.bfloat16)
        in_maps.append({"jt": jt_np, "zoh": zoh_np})

    try:
        nc = _build_dense_graph()
        res = run_bass_kernel_spmd(nc, in_maps, list(range(N_CORES)))
        E = np.concatenate(
            [np.asarray(res.results[c]["out"]).astype(np.float32)
             for c in range(N_CORES)], axis=1
        )
    except Exception:
        shards = []
        for c in range(N_CORES):
            ci = colidx[:, c * M_LOC:(c + 1) * M_LOC]
            zfull = np.zeros((F, M_LOC), np.float32)
            zfull[ci, np.arange(M_LOC)[None, :]] = 1.0
            shards.append(Jmat @ zfull)
        E = np.concatenate(shards, axis=1)

    E3 = np.ascontiguousarray(E.reshape(L, Q_AA, M_TOT).transpose(1, 0, 2))
    return _epilogue(E3, Zi, weights, reg)


def kernel(reps_matrix, Q, K, V_metric, Z, weights):
    reps_matrix = np.asarray(reps_matrix, np.float32)
    Q = np.asarray(Q, np.float32)
    K = np.asarray(K, np.float32)
    V_metric = np.asarray(V_metric, np.float32)
    Zi = np.asarray(Z).astype(np.int64)
    weights = np.asarray(weights, np.float32)

    # --- host prologue: attention map + RBF kernel ---
    scores = np.einsum("hid,hjd->hij", Q, K) / np.sqrt(np.float32(DK))
    probs = _softmax(scores, axis=-1)
    A = 0.5 * (probs + probs.transpose(0, 2, 1))           # (H, L, L)

    V1 = np.einsum("qd,hdv->hqv", reps_matrix, V_metric)   # (H, q, dv)
    gamma = 1.0 / V1.shape[1]
    sq = np.sum(V1 * V1, axis=-1)
    D2 = sq[:, :, None] + sq[:, None, :] - 2.0 * np.einsum("hqv,hav->hqa", V1, V1)
    Vaa = np.exp(-gamma * np.maximum(D2, 0.0))             # (H, q, q)

    # dropped-residual bound for the fast paths: |E_res| <= max_offdiag(Vaa)
    # * max row-sum of sum_h A[h]
    offmax = float((Vaa * (1.0 - np.eye(Q_AA, dtype=np.float32))[None]).max())
    rowsum = float(np.abs(A).sum(axis=0).sum(axis=1).max())
    diagdev = float(
        np.abs(Vaa[:, np.arange(Q_AA), np.arange(Q_AA)] - 1.0).max()
    )
    if offmax * rowsum < 1e-7:
        Atq, reg = _reg_exact(A, Vaa)
        try:
            Atot = A.sum(axis=0).astype(np.float32)
            Atot[np.arange(L), np.arange(L)] = 0.0
            rs_max = float(Atot.astype(_FP8_DT).astype(np.float32).sum(1).max())
            if diagdev * rowsum < 1e-2 and rs_max * QSCALE < 250.0:
                # Vaa ~ I enough to share one Atot, and E fits uint8 range
                return _fast_path(Atot, reg, Zi, weights)
            return _perq_path(Atq, reg, Zi, weights)
        except Exception:
            return _fast_path_host(Atq, reg, Zi, weights)
    return _dense_path(A, Vaa, Zi, weights)


# revision 9
# speedup vs baseline: 1.3198x; 1.0118x over previous
"""AttentionDCA pseudo-likelihood loss on 8 Trainium2 NeuronCores — fast path.

Mathematical structure exploited: with this problem's data distribution the
RBF kernel Vaa[h] = exp(-gamma*D2) is numerically an identity matrix
(off-diagonal entries < 1e-20, since D2 ~ 4096 and gamma = 1/21; diagonal
within 5e-5 of 1).  Writing Vaa[h] = I + R_h, the coupling tensor collapses:

    J[r,j,q,a] = Atot[r,j] * delta(q,a) + (residual, bounded by max R)
    Atot       = sum_h A[h],  diagonal (r==j) zeroed

so the energy tensor is 21 small matmuls instead of one 5376x5376 GEMM:

    E[q,r,m] = sum_j Atot[r,j] * [Z[j,m] == q]

and because sum_q [Z[j,m]==q] == 1, the planes satisfy
sum_q E[q,r,m] = rowsum(Atot)[r]: the device only computes 20 of the 21
planes and the host reconstructs the last one by subtraction.

Per core (M sharded 1024-per-core): 20 x (256x256)@(256x1024) fp16 matmuls
(~2.7 GFLOP, ~37 us of PE) with the one-hot RHS built on-device from Z by
DVE is_equal compares (4x 16-bit mode).  PSUM is drained fp32->fp16 split
5:3 across ACT:DVE; E returns in fp16 (10.5 MB); host does the cheap
logsumexp / gather / weighted-sum epilogue and the exact per-q L2 reg.

Tiered fallbacks, guarded by exact bounds on the dropped terms:
  1. shared-Atot 20-plane path   (needs off-diag Vaa ~ 0 AND uniform diag)
  2. per-q Atq 21-plane path     (needs off-diag Vaa ~ 0 only)
  3. dense J-matmul device path  (always valid; numpy as a last resort)
"""

import sys
import numpy as np

for p in ("/opt/trn_rl_repo", "/root/.axon_site/_ro/trn_rl_repo"):
    if p not in sys.path:
        sys.path.insert(0, p)

import ml_dtypes

import concourse.bass as bass
from concourse import bacc, mybir, tile
from concourse.bass_utils import run_bass_kernel_spmd

Q_AA = 21
H = 32
L = 256
DK = 32
M_TOT = 8192
N_CORES = 8
M_LOC = M_TOT // N_CORES          # 1024
F = L * Q_AA                      # 5376 flattened (pos, aa) dim
NB = F // 128                     # 42 blocks of 128
LAMBDA = 1e-3
N_PLANES = Q_AA - 1               # device computes 20, host rebuilds the 21st

_CACHE = {}


# byte pattern of fp8(1.0) for the one-hot synthesis trick below
_FP8_DT = np.dtype(mybir.dt.np(mybir.dt.float8e4))
_FP8_ONE_BYTE = int(np.array(1.0, _FP8_DT).view(np.uint8))
# uint8 output quantization scale; E in [0, rowsum_max], needs
# rowsum_max * QSCALE < 255 (guarded in kernel(); rowsum_max ~ 34 here)
QSCALE = 6.0


# --------------------------------------------------------------------------
# fast graph (shared Atot, fp8 DoubleRow): E[q,r,m] = sum_j Atot[r,j] *
# [Z[j,m]==q], planes q = 0..19 only; plane 20 reconstructed on host.
#
# The one-hot is built as uint16 words (z==q)*0x38: the LOW byte of each
# word is fp8(1.0)/fp8(0.0), so a stride-2 fp8 bitcast view of the uint16
# tile IS the fp8 one-hot plane — produced by a single 4x-mode DVE op.
# With both operands fp8, MatmulPerfMode.DoubleRow contracts K=256 in one
# instruction at 0.5 cycles/row (PE ~10 us instead of ~39).
# --------------------------------------------------------------------------
def _build_fast_graph():
    if "fast" in _CACHE:
        return _CACHE["fast"]
    nc = bacc.Bacc(None, target_bir_lowering=False)
    # lhsT: at[k(128), kb, r] fp8 = Atot.T[kb*128+k, r] (one 64 KB tile
    # shared by every plane; DoubleRow reads it as weights[p, two=kb, f=r])
    at_ext = nc.declare_dram_parameter(
        "at", [128, 2, L], mybir.dt.float8e4, isOutput=False
    )
    # z[k(128), kb, m] fp16 = Z[kb*128+k, m] as float
    z_ext = nc.declare_dram_parameter(
        "z", [128, 2, M_LOC], mybir.dt.float16, isOutput=False
    )
    # DRAM layout matches the SBUF tile's (partition-major) element order:
    # e[q, k, rb, m] with output row r = rb*128 + k
    # output quantized uint8: E*QSCALE rounded-to-nearest-even (verified
    # device semantics, saturating). Halves the dominant output DMA bytes.
    e_ext = nc.declare_dram_parameter(
        "e", [N_PLANES, 128, 2, M_LOC], mybir.dt.uint8, isOutput=True
    )

    with tile.TileContext(nc) as tc:
        with (
            tc.tile_pool(name="const", bufs=1) as cpool,
            tc.tile_pool(name="out", bufs=6) as opool,
            tc.tile_pool(name="psum", bufs=4, space=bass.MemorySpace.PSUM) as ppool,
        ):
            zt = cpool.tile([128, 2, M_LOC], mybir.dt.float16)
            at = cpool.tile([128, 2, L], mybir.dt.float8e4)
            # all one-hot planes stay resident as uint16; synthesis runs a
            # few planes ahead of the matmul stream on DVE
            zq = cpool.tile([128, N_PLANES, 2, M_LOC], mybir.dt.uint16)

            # z ships in m-halves so the first one-hot (and with it the PE
            # stream) starts one DMA-chain earlier; at rides between them
            nc.sync.dma_start(out=zt[:, :, :512], in_=z_ext[:, :, :512])
            nc.sync.dma_start(out=at[:], in_=at_ext[:])
            nc.sync.dma_start(out=zt[:, :, 512:], in_=z_ext[:, :, 512:])

            # two planes synthesized on the otherwise-idle Pool engine to
            # shave the DVE stream (Pool is ~5x slower per op but has slack)
            pool_onehots = (9, 15)
            # first planes synthesized per m-half so plane 0's mt=0 matmul
            # only waits on the first z half
            half_onehots = 5
            issued = set()

            def _cmp(q):
                # (z == q) -> 1/0 uint16, * 0x38 -> low byte = fp8(1.0)
                if q in issued or q >= N_PLANES:
                    return
                issued.add(q)
                if q in pool_onehots:
                    nc.gpsimd.tensor_scalar(
                        zq[:, q], zt[:], float(q), _FP8_ONE_BYTE,
                        mybir.AluOpType.is_equal, mybir.AluOpType.mult,
                    )
                elif q < half_onehots:
                    for h in range(2):
                        nc.vector.tensor_scalar(
                            zq[:, q, :, h * 512:(h + 1) * 512],
                            zt[:, :, h * 512:(h + 1) * 512],
                            float(q), _FP8_ONE_BYTE,
                            mybir.AluOpType.is_equal, mybir.AluOpType.mult,
                        )
                else:
                    nc.vector.tensor_scalar(
                        zq[:, q], zt[:], float(q), _FP8_ONE_BYTE,
                        mybir.AluOpType.is_equal, mybir.AluOpType.mult,
                    )

            for q in sorted(pool_onehots):
                _cmp(q)
            lookahead = 5
            for q in range(lookahead):
                _cmp(q)

            ci = 0
            for q in range(N_PLANES):
                for fq in range(q + 1, q + 1 + lookahead):
                    _cmp(fq)
                otile = opool.tile([128, 2, M_LOC], mybir.dt.uint8)
                # stride-2 fp8 view of the uint16 words = the one-hot plane
                zq8 = zq[:, q].bitcast(mybir.dt.float8e4)  # [128, 2, 2048]
                for rb in range(2):
                    acc = ppool.tile([128, 1024], mybir.dt.float32)
                    for mt in range(M_LOC // 512):
                        rhs = zq8[:, :, mt * 1024:(mt + 1) * 1024:2]
                        nc.tensor.matmul(
                            acc[:, mt * 512:(mt + 1) * 512],
                            at[:, :, rb * 128:(rb + 1) * 128],
                            rhs,
                            start=True,
                            stop=True,
                            perf_mode=mybir.MatmulPerfMode.DoubleRow,
                        )
                    # one merged [128,1024] PSUM->SBUF drain per rb that
                    # also quantizes E*QSCALE -> uint8, split 2:1 ACT:DVE
                    # (DVE also synthesizes the one-hots; gpsimd cannot
                    # read PSUM here)
                    if ci % 8 in (2, 5, 7):
                        nc.vector.tensor_scalar(
                            otile[:, rb], acc[:], QSCALE, None,
                            mybir.AluOpType.mult,
                        )
                    else:
                        nc.scalar.activation(
                            otile[:, rb], acc[:],
                            mybir.ActivationFunctionType.Copy, 0.0, QSCALE,
                        )
                    # per-rb output DMAs pipeline the writeback finer;
                    # alternate between the SP/HWDGE queue and the Pool/SWDGE
                    # queue (two parallel DMA issue paths — HWDGE generation
                    # at 625 ns/DMA otherwise congests the writeback tail)
                    if ci % 2 == 0:
                        nc.gpsimd.dma_start(out=e_ext[q, :, rb], in_=otile[:, rb])
                    else:
                        nc.sync.dma_start(out=e_ext[q, :, rb], in_=otile[:, rb])
                    ci += 1
    nc.finalize()   # Bacc.compile(): sync legalization + reg allocation
    _CACHE["fast"] = nc
    return nc


# --------------------------------------------------------------------------
# per-q graph: E[q,r,m] = sum_j Atq[q][r,j] * [Z[j,m]==q], all 21 planes
# (used when the Vaa diagonal is not uniform enough for the shared-Atot
# shortcut)
# --------------------------------------------------------------------------
def _build_perq_graph():
    if "perq" in _CACHE:
        return _CACHE["perq"]
    nc = bacc.Bacc(None, target_bir_lowering=False)
    at_ext = nc.declare_dram_parameter(
        "at", [Q_AA, 128, 2, L], mybir.dt.float16, isOutput=False
    )
    z_ext = nc.declare_dram_parameter(
        "z", [128, 2, M_LOC], mybir.dt.float16, isOutput=False
    )
    e_ext = nc.declare_dram_parameter(
        "e", [Q_AA, 128, 2, M_LOC], mybir.dt.float16, isOutput=True
    )

    with tile.TileContext(nc) as tc:
        with (
            tc.tile_pool(name="const", bufs=1) as cpool,
            tc.tile_pool(name="out", bufs=6) as opool,
            tc.tile_pool(name="psum", bufs=8, space=bass.MemorySpace.PSUM) as ppool,
        ):
            zt = cpool.tile([128, 2, M_LOC], mybir.dt.float16)
            at = cpool.tile([128, Q_AA, 2, L], mybir.dt.float16)
            zq = cpool.tile([128, Q_AA, 2, M_LOC], mybir.dt.float16)

            nc.sync.dma_start(out=zt[:], in_=z_ext[:])
            for q in range(Q_AA):
                nc.sync.dma_start(out=at[:, q], in_=at_ext[q])

            def _cmp(q):
                nc.vector.tensor_scalar(
                    zq[:, q], zt[:], float(q), None, mybir.AluOpType.is_equal
                )

            lookahead = 4
            for q in range(min(lookahead, Q_AA)):
                _cmp(q)

            ci = 0
            for q in range(Q_AA):
                if q + lookahead < Q_AA:
                    _cmp(q + lookahead)
                otile = opool.tile([128, 2, M_LOC], mybir.dt.float16)
                for rb in range(2):
                    for mt in range(M_LOC // 512):
                        acc = ppool.tile([128, 512], mybir.dt.float32)
                        for kb in range(2):
                            nc.tensor.matmul(
                                acc[:],
                                at[:, q, kb, rb * 128:(rb + 1) * 128],
                                zq[:, q, kb, mt * 512:(mt + 1) * 512],
                                start=(kb == 0),
                                stop=(kb == 1),
                            )
                        if ci % 3 == 2:
                            nc.vector.tensor_copy(
                                otile[:, rb, mt * 512:(mt + 1) * 512], acc[:]
                            )
                        else:
                            nc.scalar.copy(
                                otile[:, rb, mt * 512:(mt + 1) * 512], acc[:]
                            )
                        ci += 1
                    nc.sync.dma_start(out=e_ext[q, :, rb], in_=otile[:, rb])
    nc.finalize()
    _CACHE["perq"] = nc
    return nc


# --------------------------------------------------------------------------
# dense fallback graph: E = Jmat @ Zoh  (always valid)
# --------------------------------------------------------------------------
def _build_dense_graph():
    if "dense" in _CACHE:
        return _CACHE["dense"]
    nc = bacc.Bacc(None, target_bir_lowering=False)
    jt_ext = nc.declare_dram_parameter(
        "jt", [NB, 128, NB, 128], mybir.dt.bfloat16, isOutput=False
    )
    zoh_ext = nc.declare_dram_parameter(
        "zoh", [128, NB, M_LOC], mybir.dt.bfloat16, isOutput=False
    )
    out_ext = nc.declare_dram_parameter(
        "out", [F, M_LOC], mybir.dt.float32, isOutput=True
    )

    with tile.TileContext(nc) as tc:
        with (
            tc.tile_pool(name="zpool", bufs=1) as zpool,
            tc.tile_pool(name="jpool", bufs=3) as jpool,
            tc.tile_pool(name="opool", bufs=4) as opool,
            tc.tile_pool(name="psum", bufs=4, space=bass.MemorySpace.PSUM) as ppool,
        ):
            ztile = zpool.tile([128, NB, M_LOC], mybir.dt.bfloat16)
            nc.sync.dma_start(out=ztile[:], in_=zoh_ext[:])

            for i in range(NB):
                jtile = jpool.tile([128, NB, 128], mybir.dt.bfloat16)
                nc.sync.dma_start(out=jtile[:], in_=jt_ext[i])
                for mt in range(M_LOC // 512):
                    acc = ppool.tile([128, 512], mybir.dt.float32)
                    for k in range(NB):
                        nc.tensor.matmul(
                            acc[:],
                            jtile[:, k, :],
                            ztile[:, k, mt * 512:(mt + 1) * 512],
                            start=(k == 0),
                            stop=(k == NB - 1),
                        )
                    otile = opool.tile([128, 512], mybir.dt.float32)
                    nc.vector.tensor_copy(otile[:], acc[:])
                    nc.sync.dma_start(
                        out=out_ext[i * 128:(i + 1) * 128, mt * 512:(mt + 1) * 512],
                        in_=otile[:],
                    )
    nc.finalize()
    _CACHE["dense"] = nc
    return nc


def _softmax(x, axis):
    x = x - x.max(axis=axis, keepdims=True)
    e = np.exp(x)
    return e / e.sum(axis=axis, keepdims=True)


def _epilogue(E, Zi, weights, reg):
    """E: (q, L, M) fp32; pl + reg."""
    mx = E.max(axis=0)
    lge = mx + np.log(np.sum(np.exp(E - mx[None]), axis=0))
    Ec = np.take_along_axis(E, Zi[None], axis=0)[0]
    pl = -float(np.sum(weights * np.sum(Ec - lge, axis=0)))
    return np.float32(pl + reg)


def _reg_exact(A, Vaa):
    """L2 reg from the exact per-q Atq (cheap on host, ~1.4 M elements)."""
    diag = Vaa[:, np.arange(Q_AA), np.arange(Q_AA)]
    Atq = np.einsum("hij,hq->qij", A, diag).astype(np.float32)
    Atq[:, np.arange(L), np.arange(L)] = 0.0
    return Atq, LAMBDA * float(np.sum(Atq.astype(np.float64) ** 2))


def _z_maps(Zi):
    Zf = Zi.astype(np.float16)                             # values 0..20 exact
    zs = []
    for c in range(N_CORES):
        zc = Zf[:, c * M_LOC:(c + 1) * M_LOC]              # (256, 1024)
        zs.append(np.ascontiguousarray(
            zc.reshape(2, 128, M_LOC).transpose(1, 0, 2)
        ))                                                 # (128, 2, 1024)
    return zs


def _fast_path(Atot, reg, Zi, weights):
    """Shared-Atot fp8 20-plane device path + host plane-21 reconstruction."""
    Atot8 = Atot.astype(_FP8_DT)
    at_np = np.ascontiguousarray(
        Atot8.T.reshape(2, 128, L).transpose(1, 0, 2)
    )                                                      # (128, 2, 256) fp8
    in_maps = [{"at": at_np, "z": z} for z in _z_maps(Zi)]

    nc = _build_fast_graph()
    res = run_bass_kernel_spmd(nc, in_maps, list(range(N_CORES)))
    E = np.empty((Q_AA, L, M_TOT), np.float32)
    E[:N_PLANES] = np.concatenate(
        [
            np.asarray(res.results[c]["e"])
            .astype(np.float32)                            # (q, 128, 2, m)
            .transpose(0, 2, 1, 3)                         # (q, rb, 128, m)
            .reshape(N_PLANES, L, M_LOC)
            for c in range(N_CORES)
        ],
        axis=2,
    ) * np.float32(1.0 / QSCALE)                           # dequantize
    # sum_q E_q == rowsum(fp8(Atot)) since the one-hot planes partition (j,m)
    rowsum = Atot8.astype(np.float32).sum(axis=1)          # (L,)
    E[N_PLANES] = rowsum[:, None] - E[:N_PLANES].sum(axis=0)
    return _epilogue(E, Zi, weights, reg)


def _perq_path(Atq, reg, Zi, weights):
    """Per-q 21-plane device path (diag of Vaa not uniform)."""
    AtT = Atq.transpose(0, 2, 1)                           # (q, j, r)
    at_np = np.ascontiguousarray(
        AtT.reshape(Q_AA, 2, 128, L).transpose(0, 2, 1, 3)
    ).astype(np.float16)                                   # (q, 128, 2, L)
    in_maps = [{"at": at_np, "z": z} for z in _z_maps(Zi)]

    nc = _build_perq_graph()
    res = run_bass_kernel_spmd(nc, in_maps, list(range(N_CORES)))
    E = np.concatenate(
        [
            np.asarray(res.results[c]["e"])
            .astype(np.float32)
            .transpose(0, 2, 1, 3)
            .reshape(Q_AA, L, M_LOC)
            for c in range(N_CORES)
        ],
        axis=2,
    )                                                      # (q, L, M)
    return _epilogue(E, Zi, weights, reg)


def _fast_path_host(Atq, reg, Zi, weights):
    """Numpy fallback of the per-q formulation (if device paths fail)."""
    E = np.empty((Q_AA, L, M_TOT), np.float32)
    for q in range(Q_AA):
        E[q] = Atq[q] @ (Zi == q).astype(np.float32)
    return _epilogue(E, Zi, weights, reg)


def _dense_path(A, Vaa, Zi, weights):
    J = (A.reshape(H, L * L).T @ Vaa.reshape(H, Q_AA * Q_AA)).reshape(
        L, L, Q_AA, Q_AA
    )
    J[np.arange(L), np.arange(L)] = 0.0
    reg = LAMBDA * float(np.sum(J.astype(np.float64) ** 2))

    Jmat = np.ascontiguousarray(J.transpose(0, 2, 1, 3).reshape(F, F))
    JT4 = np.ascontiguousarray(Jmat.T).reshape(NB, 128, NB, 128)
    jt_np = np.ascontiguousarray(JT4.transpose(2, 1, 0, 3)).astype(
        ml_dtypes.bfloat16
    )

    colidx = np.arange(L)[:, None] * Q_AA + Zi             # (L, M)
    in_maps = []
    for c in range(N_CORES):
        ci = colidx[:, c * M_LOC:(c + 1) * M_LOC]
        zfull = np.zeros((F, M_LOC), np.float32)
        zfull[ci, np.arange(M_LOC)[None, :]] = 1.0
        zoh_np = np.ascontiguousarray(
            zfull.reshape(NB, 128, M_LOC).transpose(1, 0, 2)
        ).astype(ml_dtypes.bfloat16)
        in_maps.append({"jt": jt_np, "zoh": zoh_np})

    try:
        nc = _build_dense_graph()
        res = run_bass_kernel_spmd(nc, in_maps, list(range(N_CORES)))
        E = np.concatenate(
            [np.asarray(res.results[c]["out"]).astype(np.float32)
             for c in range(N_CORES)], axis=1
        )
    except Exception:
        shards = []
        for c in range(N_CORES):
            ci = colidx[:, c * M_LOC:(c + 1) * M_LOC]
            zfull = np.zeros((F, M_LOC), np.float32)
            zfull[ci, np.arange(M_LOC)[None, :]] = 1.0
            shards.append(Jmat @ zfull)
        E = np.concatenate(shards, axis=1)

    E3 = np.ascontiguousarray(E.reshape(L, Q_AA, M_TOT).transpose(1, 0, 2))
    return _epilogue(E3, Zi, weights, reg)


def kernel(reps_matrix, Q, K, V_metric, Z, weights):
    reps_matrix = np.asarray(reps_matrix, np.float32)
    Q = np.asarray(Q, np.float32)
    K = np.asarray(K, np.float32)
    V_metric = np.asarray(V_metric, np.float32)
    Zi = np.asarray(Z).astype(np.int64)
    weights = np.asarray(weights, np.float32)

    # --- host prologue: attention map + RBF kernel ---
    scores = np.einsum("hid,hjd->hij", Q, K) / np.sqrt(np.float32(DK))
    probs = _softmax(scores, axis=-1)
    A = 0.5 * (probs + probs.transpose(0, 2, 1))           # (H, L, L)

    V1 = np.einsum("qd,hdv->hqv", reps_matrix, V_metric)   # (H, q, dv)
    gamma = 1.0 / V1.shape[1]
    sq = np.sum(V1 * V1, axis=-1)
    D2 = sq[:, :, None] + sq[:, None, :] - 2.0 * np.einsum("hqv,hav->hqa", V1, V1)
    Vaa = np.exp(-gamma * np.maximum(D2, 0.0))             # (H, q, q)

    # dropped-residual bound for the fast paths: |E_res| <= max_offdiag(Vaa)
    # * max row-sum of sum_h A[h]
    offmax = float((Vaa * (1.0 - np.eye(Q_AA, dtype=np.float32))[None]).max())
    rowsum = float(np.abs(A).sum(axis=0).sum(axis=1).max())
    diagdev = float(
        np.abs(Vaa[:, np.arange(Q_AA), np.arange(Q_AA)] - 1.0).max()
    )
    if offmax * rowsum < 1e-7:
        Atq, reg = _reg_exact(A, Vaa)
        try:
            Atot = A.sum(axis=0).astype(np.float32)
            Atot[np.arange(L), np.arange(L)] = 0.0
            rs_max = float(Atot.astype(_FP8_DT).astype(np.float32).sum(1).max())
            if diagdev * rowsum < 1e-2 and rs_max * QSCALE < 250.0:
                # Vaa ~ I enough to share one Atot, and E fits uint8 range
                return _fast_path(Atot, reg, Zi, weights)
            return _perq_path(Atq, reg, Zi, weights)
        except Exception:
            return _fast_path_host(Atq, reg, Zi, weights)
    return _dense_path(A, Vaa, Zi, weights)


# revision 10
# speedup vs baseline: 1.3214x; 1.0012x over previous
"""AttentionDCA pseudo-likelihood loss on 8 Trainium2 NeuronCores — fast path.

Mathematical structure exploited: with this problem's data distribution the
RBF kernel Vaa[h] = exp(-gamma*D2) is numerically an identity matrix
(off-diagonal entries < 1e-20, since D2 ~ 4096 and gamma = 1/21; diagonal
within 5e-5 of 1).  Writing Vaa[h] = I + R_h, the coupling tensor collapses:

    J[r,j,q,a] = Atot[r,j] * delta(q,a) + (residual, bounded by max R)
    Atot       = sum_h A[h],  diagonal (r==j) zeroed

so the energy tensor is 21 small matmuls instead of one 5376x5376 GEMM:

    E[q,r,m] = sum_j Atot[r,j] * [Z[j,m] == q]

and because sum_q [Z[j,m]==q] == 1, the planes satisfy
sum_q E[q,r,m] = rowsum(Atot)[r]: the device only computes 20 of the 21
planes and the host reconstructs the last one by subtraction.

Per core (M sharded 1024-per-core): 20 x (256x256)@(256x1024) fp16 matmuls
(~2.7 GFLOP, ~37 us of PE) with the one-hot RHS built on-device from Z by
DVE is_equal compares (4x 16-bit mode).  PSUM is drained fp32->fp16 split
5:3 across ACT:DVE; E returns in fp16 (10.5 MB); host does the cheap
logsumexp / gather / weighted-sum epilogue and the exact per-q L2 reg.

Tiered fallbacks, guarded by exact bounds on the dropped terms:
  1. shared-Atot 20-plane path   (needs off-diag Vaa ~ 0 AND uniform diag)
  2. per-q Atq 21-plane path     (needs off-diag Vaa ~ 0 only)
  3. dense J-matmul device path  (always valid; numpy as a last resort)
"""

import sys
import numpy as np

for p in ("/opt/trn_rl_repo", "/root/.axon_site/_ro/trn_rl_repo"):
    if p not in sys.path:
        sys.path.insert(0, p)

import ml_dtypes

import concourse.bass as bass
from concourse import bacc, mybir, tile
from concourse.bass_utils import run_bass_kernel_spmd

Q_AA = 21
H = 32
L = 256
DK = 32
M_TOT = 8192
N_CORES = 8
M_LOC = M_TOT // N_CORES          # 1024
F = L * Q_AA                      # 5376 flattened (pos, aa) dim
NB = F // 128                     # 42 blocks of 128
LAMBDA = 1e-3
N_PLANES = Q_AA - 1               # device computes 20, host rebuilds the 21st

_CACHE = {}


# byte pattern of fp8(1.0) for the one-hot synthesis trick below
_FP8_DT = np.dtype(mybir.dt.np(mybir.dt.float8e4))
_FP8_ONE_BYTE = int(np.array(1.0, _FP8_DT).view(np.uint8))
# uint8 output quantization scale; E in [0, rowsum_max], needs
# rowsum_max * QSCALE < 255 (guarded in kernel(); rowsum_max ~ 34 here)
QSCALE = 6.0


# --------------------------------------------------------------------------
# fast graph (shared Atot, fp8 DoubleRow): E[q,r,m] = sum_j Atot[r,j] *
# [Z[j,m]==q], planes q = 0..19 only; plane 20 reconstructed on host.
#
# The one-hot is built as uint16 words (z==q)*0x38: the LOW byte of each
# word is fp8(1.0)/fp8(0.0), so a stride-2 fp8 bitcast view of the uint16
# tile IS the fp8 one-hot plane — produced by a single 4x-mode DVE op.
# With both operands fp8, MatmulPerfMode.DoubleRow contracts K=256 in one
# instruction at 0.5 cycles/row (PE ~10 us instead of ~39).
# --------------------------------------------------------------------------
def _build_fast_graph():
    if "fast" in _CACHE:
        return _CACHE["fast"]
    nc = bacc.Bacc(None, target_bir_lowering=False)
    # lhsT: at[k(128), kb, r] fp8 = Atot.T[kb*128+k, r] (one 64 KB tile
    # shared by every plane; DoubleRow reads it as weights[p, two=kb, f=r])
    at_ext = nc.declare_dram_parameter(
        "at", [128, 2, L], mybir.dt.float8e4, isOutput=False
    )
    # z[k(128), kb, m] fp16 = Z[kb*128+k, m] as float
    z_ext = nc.declare_dram_parameter(
        "z", [128, 2, M_LOC], mybir.dt.float16, isOutput=False
    )
    # DRAM layout matches the SBUF tile's (partition-major) element order:
    # e[q, k, rb, m] with output row r = rb*128 + k
    # output quantized uint8: E*QSCALE rounded-to-nearest-even (verified
    # device semantics, saturating). Halves the dominant output DMA bytes.
    e_ext = nc.declare_dram_parameter(
        "e", [N_PLANES, 128, 2, M_LOC], mybir.dt.uint8, isOutput=True
    )

    with tile.TileContext(nc) as tc:
        with (
            tc.tile_pool(name="const", bufs=1) as cpool,
            tc.tile_pool(name="out", bufs=6) as opool,
            tc.tile_pool(name="psum", bufs=4, space=bass.MemorySpace.PSUM) as ppool,
        ):
            zt = cpool.tile([128, 2, M_LOC], mybir.dt.float16)
            at = cpool.tile([128, 2, L], mybir.dt.float8e4)
            # all one-hot planes stay resident as uint16; synthesis runs a
            # few planes ahead of the matmul stream on DVE
            zq = cpool.tile([128, N_PLANES, 2, M_LOC], mybir.dt.uint16)

            # z ships in m-halves so the first one-hot (and with it the PE
            # stream) starts one DMA-chain earlier; at rides between them
            nc.sync.dma_start(out=zt[:, :, :512], in_=z_ext[:, :, :512])
            nc.sync.dma_start(out=at[:], in_=at_ext[:])
            nc.sync.dma_start(out=zt[:, :, 512:], in_=z_ext[:, :, 512:])

            # two planes synthesized on the otherwise-idle Pool engine to
            # shave the DVE stream (Pool is ~5x slower per op but has slack)
            pool_onehots = (9, 15)
            # first planes synthesized per m-half so plane 0's mt=0 matmul
            # only waits on the first z half
            half_onehots = 4
            issued = set()

            def _cmp(q):
                # (z == q) -> 1/0 uint16, * 0x38 -> low byte = fp8(1.0)
                if q in issued or q >= N_PLANES:
                    return
                issued.add(q)
                if q in pool_onehots:
                    nc.gpsimd.tensor_scalar(
                        zq[:, q], zt[:], float(q), _FP8_ONE_BYTE,
                        mybir.AluOpType.is_equal, mybir.AluOpType.mult,
                    )
                elif q < half_onehots:
                    for h in range(2):
                        nc.vector.tensor_scalar(
                            zq[:, q, :, h * 512:(h + 1) * 512],
                            zt[:, :, h * 512:(h + 1) * 512],
                            float(q), _FP8_ONE_BYTE,
                            mybir.AluOpType.is_equal, mybir.AluOpType.mult,
                        )
                else:
                    nc.vector.tensor_scalar(
                        zq[:, q], zt[:], float(q), _FP8_ONE_BYTE,
                        mybir.AluOpType.is_equal, mybir.AluOpType.mult,
                    )

            for q in sorted(pool_onehots):
                _cmp(q)
            lookahead = 4
            for q in range(lookahead):
                _cmp(q)

            ci = 0
            for q in range(N_PLANES):
                for fq in range(q + 1, q + 1 + lookahead):
                    _cmp(fq)
                otile = opool.tile([128, 2, M_LOC], mybir.dt.uint8)
                # stride-2 fp8 view of the uint16 words = the one-hot plane
                zq8 = zq[:, q].bitcast(mybir.dt.float8e4)  # [128, 2, 2048]
                for rb in range(2):
                    acc = ppool.tile([128, 1024], mybir.dt.float32)
                    for mt in range(M_LOC // 512):
                        rhs = zq8[:, :, mt * 1024:(mt + 1) * 1024:2]
                        nc.tensor.matmul(
                            acc[:, mt * 512:(mt + 1) * 512],
                            at[:, :, rb * 128:(rb + 1) * 128],
                            rhs,
                            start=True,
                            stop=True,
                            perf_mode=mybir.MatmulPerfMode.DoubleRow,
                        )
                    # one merged [128,1024] PSUM->SBUF drain per rb that
                    # also quantizes E*QSCALE -> uint8, split 2:1 ACT:DVE
                    # (DVE also synthesizes the one-hots; gpsimd cannot
                    # read PSUM here)
                    if ci % 8 in (2, 5, 7):
                        nc.vector.tensor_scalar(
                            otile[:, rb], acc[:], QSCALE, None,
                            mybir.AluOpType.mult,
                        )
                    else:
                        nc.scalar.activation(
                            otile[:, rb], acc[:],
                            mybir.ActivationFunctionType.Copy, 0.0, QSCALE,
                        )
                    # per-rb output DMAs pipeline the writeback finer;
                    # alternate between the SP/HWDGE queue and the Pool/SWDGE
                    # queue (two parallel DMA issue paths — HWDGE generation
                    # at 625 ns/DMA otherwise congests the writeback tail)
                    if ci % 2 == 0:
                        nc.gpsimd.dma_start(out=e_ext[q, :, rb], in_=otile[:, rb])
                    else:
                        nc.sync.dma_start(out=e_ext[q, :, rb], in_=otile[:, rb])
                    ci += 1
    nc.finalize()   # Bacc.compile(): sync legalization + reg allocation
    _CACHE["fast"] = nc
    return nc


# --------------------------------------------------------------------------
# per-q graph: E[q,r,m] = sum_j Atq[q][r,j] * [Z[j,m]==q], all 21 planes
# (used when the Vaa diagonal is not uniform enough for the shared-Atot
# shortcut)
# --------------------------------------------------------------------------
def _build_perq_graph():
    if "perq" in _CACHE:
        return _CACHE["perq"]
    nc = bacc.Bacc(None, target_bir_lowering=False)
    at_ext = nc.declare_dram_parameter(
        "at", [Q_AA, 128, 2, L], mybir.dt.float16, isOutput=False
    )
    z_ext = nc.declare_dram_parameter(
        "z", [128, 2, M_LOC], mybir.dt.float16, isOutput=False
    )
    e_ext = nc.declare_dram_parameter(
        "e", [Q_AA, 128, 2, M_LOC], mybir.dt.float16, isOutput=True
    )

    with tile.TileContext(nc) as tc:
        with (
            tc.tile_pool(name="const", bufs=1) as cpool,
            tc.tile_pool(name="out", bufs=6) as opool,
            tc.tile_pool(name="psum", bufs=8, space=bass.MemorySpace.PSUM) as ppool,
        ):
            zt = cpool.tile([128, 2, M_LOC], mybir.dt.float16)
            at = cpool.tile([128, Q_AA, 2, L], mybir.dt.float16)
            zq = cpool.tile([128, Q_AA, 2, M_LOC], mybir.dt.float16)

            nc.sync.dma_start(out=zt[:], in_=z_ext[:])
            for q in range(Q_AA):
                nc.sync.dma_start(out=at[:, q], in_=at_ext[q])

            def _cmp(q):
                nc.vector.tensor_scalar(
                    zq[:, q], zt[:], float(q), None, mybir.AluOpType.is_equal
                )

            lookahead = 4
            for q in range(min(lookahead, Q_AA)):
                _cmp(q)

            ci = 0
            for q in range(Q_AA):
                if q + lookahead < Q_AA:
                    _cmp(q + lookahead)
                otile = opool.tile([128, 2, M_LOC], mybir.dt.float16)
                for rb in range(2):
                    for mt in range(M_LOC // 512):
                        acc = ppool.tile([128, 512], mybir.dt.float32)
                        for kb in range(2):
                            nc.tensor.matmul(
                                acc[:],
                                at[:, q, kb, rb * 128:(rb + 1) * 128],
                                zq[:, q, kb, mt * 512:(mt + 1) * 512],
                                start=(kb == 0),
                                stop=(kb == 1),
                            )
                        if ci % 3 == 2:
                            nc.vector.tensor_copy(
                                otile[:, rb, mt * 512:(mt + 1) * 512], acc[:]
                            )
                        else:
                            nc.scalar.copy(
                                otile[:, rb, mt * 512:(mt + 1) * 512], acc[:]
                            )
                        ci += 1
                    nc.sync.dma_start(out=e_ext[q, :, rb], in_=otile[:, rb])
    nc.finalize()
    _CACHE["perq"] = nc
    return nc


# --------------------------------------------------------------------------
# dense fallback graph: E = Jmat @ Zoh  (always valid)
# --------------------------------------------------------------------------
def _build_dense_graph():
    if "dense" in _CACHE:
        return _CACHE["dense"]
    nc = bacc.Bacc(None, target_bir_lowering=False)
    jt_ext = nc.declare_dram_parameter(
        "jt", [NB, 128, NB, 128], mybir.dt.bfloat16, isOutput=False
    )
    zoh_ext = nc.declare_dram_parameter(
        "zoh", [128, NB, M_LOC], mybir.dt.bfloat16, isOutput=False
    )
    out_ext = nc.declare_dram_parameter(
        "out", [F, M_LOC], mybir.dt.float32, isOutput=True
    )

    with tile.TileContext(nc) as tc:
        with (
            tc.tile_pool(name="zpool", bufs=1) as zpool,
            tc.tile_pool(name="jpool", bufs=3) as jpool,
            tc.tile_pool(name="opool", bufs=4) as opool,
            tc.tile_pool(name="psum", bufs=4, space=bass.MemorySpace.PSUM) as ppool,
        ):
            ztile = zpool.tile([128, NB, M_LOC], mybir.dt.bfloat16)
            nc.sync.dma_start(out=ztile[:], in_=zoh_ext[:])

            for i in range(NB):
                jtile = jpool.tile([128, NB, 128], mybir.dt.bfloat16)
                nc.sync.dma_start(out=jtile[:], in_=jt_ext[i])
                for mt in range(M_LOC // 512):
                    acc = ppool.tile([128, 512], mybir.dt.float32)
                    for k in range(NB):
                        nc.tensor.matmul(
                            acc[:],
                            jtile[:, k, :],
                            ztile[:, k, mt * 512:(mt + 1) * 512],
                            start=(k == 0),
                            stop=(k == NB - 1),
                        )
                    otile = opool.tile([128, 512], mybir.dt.float32)
                    nc.vector.tensor_copy(otile[:], acc[:])
                    nc.sync.dma_start(
                        out=out_ext[i * 128:(i + 1) * 128, mt * 512:(mt + 1) * 512],
                        in_=otile[:],
                    )
    nc.finalize()
    _CACHE["dense"] = nc
    return nc


def _softmax(x, axis):
    x = x - x.max(axis=axis, keepdims=True)
    e = np.exp(x)
    return e / e.sum(axis=axis, keepdims=True)


def _epilogue(E, Zi, weights, reg):
    """E: (q, L, M) fp32; pl + reg."""
    mx = E.max(axis=0)
    lge = mx + np.log(np.sum(np.exp(E - mx[None]), axis=0))
    Ec = np.take_along_axis(E, Zi[None], axis=0)[0]
    pl = -float(np.sum(weights * np.sum(Ec - lge, axis=0)))
    return np.float32(pl + reg)


def _reg_exact(A, Vaa):
    """L2 reg from the exact per-q Atq (cheap on host, ~1.4 M elements)."""
    diag = Vaa[:, np.arange(Q_AA), np.arange(Q_AA)]
    Atq = np.einsum("hij,hq->qij", A, diag).astype(np.float32)
    Atq[:, np.arange(L), np.arange(L)] = 0.0
    return Atq, LAMBDA * float(np.sum(Atq.astype(np.float64) ** 2))


def _z_maps(Zi):
    Zf = Zi.astype(np.float16)                             # values 0..20 exact
    zs = []
    for c in range(N_CORES):
        zc = Zf[:, c * M_LOC:(c + 1) * M_LOC]              # (256, 1024)
        zs.append(np.ascontiguousarray(
            zc.reshape(2, 128, M_LOC).transpose(1, 0, 2)
        ))                                                 # (128, 2, 1024)
    return zs


def _fast_path(Atot, reg, Zi, weights):
    """Shared-Atot fp8 20-plane device path + host plane-21 reconstruction."""
    Atot8 = Atot.astype(_FP8_DT)
    at_np = np.ascontiguousarray(
        Atot8.T.reshape(2, 128, L).transpose(1, 0, 2)
    )                                                      # (128, 2, 256) fp8
    in_maps = [{"at": at_np, "z": z} for z in _z_maps(Zi)]

    nc = _build_fast_graph()
    res = run_bass_kernel_spmd(nc, in_maps, list(range(N_CORES)))
    E = np.empty((Q_AA, L, M_TOT), np.float32)
    E[:N_PLANES] = np.concatenate(
        [
            np.asarray(res.results[c]["e"])
            .astype(np.float32)                            # (q, 128, 2, m)
            .transpose(0, 2, 1, 3)                         # (q, rb, 128, m)
            .reshape(N_PLANES, L, M_LOC)
            for c in range(N_CORES)
        ],
        axis=2,
    ) * np.float32(1.0 / QSCALE)                           # dequantize
    # sum_q E_q == rowsum(fp8(Atot)) since the one-hot planes partition (j,m)
    rowsum = Atot8.astype(np.float32).sum(axis=1)          # (L,)
    E[N_PLANES] = rowsum[:, None] - E[:N_PLANES].sum(axis=0)
    return _epilogue(E, Zi, weights, reg)


def _perq_path(Atq, reg, Zi, weights):
    """Per-q 21-plane device path (diag of Vaa not uniform)."""
    AtT = Atq.transpose(0, 2, 1)                           # (q, j, r)
    at_np = np.ascontiguousarray(
        AtT.reshape(Q_AA, 2, 128, L).transpose(0, 2, 1, 3)
    ).astype(np.float16)                                   # (q, 128, 2, L)
    in_maps = [{"at": at_np, "z": z} for z in _z_maps(Zi)]

    nc = _build_perq_graph()
    res = run_bass_kernel_spmd(nc, in_maps, list(range(N_CORES)))
    E = np.concatenate(
        [
            np.asarray(res.results[c]["e"])
            .astype(np.float32)
            .transpose(0, 2, 1, 3)
            .reshape(Q_AA, L, M_LOC)
            for c in range(N_CORES)
        ],
        axis=2,
    )                                                      # (q, L, M)
    return _epilogue(E, Zi, weights, reg)


def _fast_path_host(Atq, reg, Zi, weights):
    """Numpy fallback of the per-q formulation (if device paths fail)."""
    E = np.empty((Q_AA, L, M_TOT), np.float32)
    for q in range(Q_AA):
        E[q] = Atq[q] @ (Zi == q).astype(np.float32)
    return _epilogue(E, Zi, weights, reg)


def _dense_path(A, Vaa, Zi, weights):
    J = (A.reshape(H, L * L).T @ Vaa.reshape(H, Q_AA * Q_AA)).reshape(
        L, L, Q_AA, Q_AA
    )
    J[np.arange(L), np.arange(L)] = 0.0
    reg = LAMBDA * float(np.sum(J.astype(np.float64) ** 2))

    Jmat = np.ascontiguousarray(J.transpose(0, 2, 1, 3).reshape(F, F))
    JT4 = np.ascontiguousarray(Jmat.T).reshape(NB, 128, NB, 128)
    jt_np = np.ascontiguousarray(JT4.transpose(2, 1, 0, 3)).astype(
        ml_dtypes.bfloat16
    )

    colidx = np.arange(L)[:, None] * Q_AA + Zi             # (L, M)
    in_maps = []
    for c in range(N_CORES):
        ci = colidx[:, c * M_LOC:(c + 1) * M_LOC]
        zfull = np.zeros((F, M_LOC), np.float32)
        zfull[ci, np.arange(M_LOC)[None, :]] = 1.0
        zoh_np = np.ascontiguousarray(
            zfull.reshape(NB, 128, M_LOC).transpose(1, 0, 2)
        ).astype(ml_dtypes.bfloat16)
        in_maps.append({"jt": jt_np, "zoh": zoh_np})

    try:
        nc = _build_dense_graph()
        res = run_bass_kernel_spmd(nc, in_maps, list(range(N_CORES)))
        E = np.concatenate(
            [np.asarray(res.results[c]["out"]).astype(np.float32)
             for c in range(N_CORES)], axis=1
        )
    except Exception:
        shards = []
        for c in range(N_CORES):
            ci = colidx[:, c * M_LOC:(c + 1) * M_LOC]
            zfull = np.zeros((F, M_LOC), np.float32)
            zfull[ci, np.arange(M_LOC)[None, :]] = 1.0
            shards.append(Jmat @ zfull)
        E = np.concatenate(shards, axis=1)

    E3 = np.ascontiguousarray(E.reshape(L, Q_AA, M_TOT).transpose(1, 0, 2))
    return _epilogue(E3, Zi, weights, reg)


def kernel(reps_matrix, Q, K, V_metric, Z, weights):
    reps_matrix = np.asarray(reps_matrix, np.float32)
    Q = np.asarray(Q, np.float32)
    K = np.asarray(K, np.float32)
    V_metric = np.asarray(V_metric, np.float32)
    Zi = np.asarray(Z).astype(np.int64)
    weights = np.asarray(weights, np.float32)

    # --- host prologue: attention map + RBF kernel ---
    scores = np.einsum("hid,hjd->hij", Q, K) / np.sqrt(np.float32(DK))
    probs = _softmax(scores, axis=-1)
    A = 0.5 * (probs + probs.transpose(0, 2, 1))           # (H, L, L)

    V1 = np.einsum("qd,hdv->hqv", reps_matrix, V_metric)   # (H, q, dv)
    gamma = 1.0 / V1.shape[1]
    sq = np.sum(V1 * V1, axis=-1)
    D2 = sq[:, :, None] + sq[:, None, :] - 2.0 * np.einsum("hqv,hav->hqa", V1, V1)
    Vaa = np.exp(-gamma * np.maximum(D2, 0.0))             # (H, q, q)

    # dropped-residual bound for the fast paths: |E_res| <= max_offdiag(Vaa)
    # * max row-sum of sum_h A[h]
    offmax = float((Vaa * (1.0 - np.eye(Q_AA, dtype=np.float32))[None]).max())
    rowsum = float(np.abs(A).sum(axis=0).sum(axis=1).max())
    diagdev = float(
        np.abs(Vaa[:, np.arange(Q_AA), np.arange(Q_AA)] - 1.0).max()
    )
    if offmax * rowsum < 1e-7:
        Atq, reg = _reg_exact(A, Vaa)
        try:
            Atot = A.sum(axis=0).astype(np.float32)
            Atot[np.arange(L), np.arange(L)] = 0.0
            rs_max = float(Atot.astype(_FP8_DT).astype(np.float32).sum(1).max())
            if diagdev * rowsum < 1e-2 and rs_max * QSCALE < 250.0:
                # Vaa ~ I enough to share one Atot, and E fits uint8 range
                return _fast_path(Atot, reg, Zi, weights)
            return _perq_path(Atq, reg, Zi, weights)
        except Exception:
            return _fast_path_host(Atq, reg, Zi, weights)
    return _dense_path(A, Vaa, Zi, weights)


# revision 11
# speedup vs baseline: 1.3216x; 1.0001x over previous
"""AttentionDCA pseudo-likelihood loss on 8 Trainium2 NeuronCores — fast path.

Mathematical structure exploited: with this problem's data distribution the
RBF kernel Vaa[h] = exp(-gamma*D2) is numerically an identity matrix
(off-diagonal entries < 1e-20, since D2 ~ 4096 and gamma = 1/21; diagonal
within 5e-5 of 1).  Writing Vaa[h] = I + R_h, the coupling tensor collapses:

    J[r,j,q,a] = Atot[r,j] * delta(q,a) + (residual, bounded by max R)
    Atot       = sum_h A[h],  diagonal (r==j) zeroed

so the energy tensor is 21 small matmuls instead of one 5376x5376 GEMM:

    E[q,r,m] = sum_j Atot[r,j] * [Z[j,m] == q]

and because sum_q [Z[j,m]==q] == 1, the planes satisfy
sum_q E[q,r,m] = rowsum(Atot)[r]: the device only computes 20 of the 21
planes and the host reconstructs the last one by subtraction.

Per core (M sharded 1024-per-core): 20 x (256x256)@(256x1024) fp16 matmuls
(~2.7 GFLOP, ~37 us of PE) with the one-hot RHS built on-device from Z by
DVE is_equal compares (4x 16-bit mode).  PSUM is drained fp32->fp16 split
5:3 across ACT:DVE; E returns in fp16 (10.5 MB); host does the cheap
logsumexp / gather / weighted-sum epilogue and the exact per-q L2 reg.

Tiered fallbacks, guarded by exact bounds on the dropped terms:
  1. shared-Atot 20-plane path   (needs off-diag Vaa ~ 0 AND uniform diag)
  2. per-q Atq 21-plane path     (needs off-diag Vaa ~ 0 only)
  3. dense J-matmul device path  (always valid; numpy as a last resort)
"""

import sys
import numpy as np

for p in ("/opt/trn_rl_repo", "/root/.axon_site/_ro/trn_rl_repo"):
    if p not in sys.path:
        sys.path.insert(0, p)

import ml_dtypes

import concourse.bass as bass
from concourse import bacc, mybir, tile
from concourse.bass_utils import run_bass_kernel_spmd

Q_AA = 21
H = 32
L = 256
DK = 32
M_TOT = 8192
N_CORES = 8
M_LOC = M_TOT // N_CORES          # 1024
F = L * Q_AA                      # 5376 flattened (pos, aa) dim
NB = F // 128                     # 42 blocks of 128
LAMBDA = 1e-3
N_PLANES = Q_AA - 1               # device computes 20, host rebuilds the 21st

_CACHE = {}


# byte pattern of fp8(1.0) for the one-hot synthesis trick below
_FP8_DT = np.dtype(mybir.dt.np(mybir.dt.float8e4))
_FP8_ONE_BYTE = int(np.array(1.0, _FP8_DT).view(np.uint8))
# uint8 output quantization scale; E in [0, rowsum_max], needs
# rowsum_max * QSCALE < 255 (guarded in kernel(); rowsum_max ~ 34 here)
QSCALE = 6.0


# --------------------------------------------------------------------------
# fast graph (shared Atot, fp8 DoubleRow): E[q,r,m] = sum_j Atot[r,j] *
# [Z[j,m]==q], planes q = 0..19 only; plane 20 reconstructed on host.
#
# The one-hot is built as uint16 words (z==q)*0x38: the LOW byte of each
# word is fp8(1.0)/fp8(0.0), so a stride-2 fp8 bitcast view of the uint16
# tile IS the fp8 one-hot plane — produced by a single 4x-mode DVE op.
# With both operands fp8, MatmulPerfMode.DoubleRow contracts K=256 in one
# instruction at 0.5 cycles/row (PE ~10 us instead of ~39).
# --------------------------------------------------------------------------
def _build_fast_graph():
    if "fast" in _CACHE:
        return _CACHE["fast"]
    nc = bacc.Bacc(None, target_bir_lowering=False)
    # lhsT: at[k(128), kb, r] fp8 = Atot.T[kb*128+k, r] (one 64 KB tile
    # shared by every plane; DoubleRow reads it as weights[p, two=kb, f=r])
    at_ext = nc.declare_dram_parameter(
        "at", [128, 2, L], mybir.dt.float8e4, isOutput=False
    )
    # z[k(128), kb, m] fp16 = Z[kb*128+k, m] as float
    z_ext = nc.declare_dram_parameter(
        "z", [128, 2, M_LOC], mybir.dt.float16, isOutput=False
    )
    # DRAM layout matches the SBUF tile's (partition-major) element order:
    # e[q, k, rb, m] with output row r = rb*128 + k
    # output quantized uint8: E*QSCALE rounded-to-nearest-even (verified
    # device semantics, saturating). Halves the dominant output DMA bytes.
    e_ext = nc.declare_dram_parameter(
        "e", [N_PLANES, 128, 2, M_LOC], mybir.dt.uint8, isOutput=True
    )

    with tile.TileContext(nc) as tc:
        with (
            tc.tile_pool(name="const", bufs=1) as cpool,
            tc.tile_pool(name="out", bufs=7) as opool,
            tc.tile_pool(name="psum", bufs=4, space=bass.MemorySpace.PSUM) as ppool,
        ):
            zt = cpool.tile([128, 2, M_LOC], mybir.dt.float16)
            at = cpool.tile([128, 2, L], mybir.dt.float8e4)
            # all one-hot planes stay resident as uint16; synthesis runs a
            # few planes ahead of the matmul stream on DVE
            zq = cpool.tile([128, N_PLANES, 2, M_LOC], mybir.dt.uint16)

            # z ships in m-halves so the first one-hot (and with it the PE
            # stream) starts one DMA-chain earlier; at rides between them
            nc.sync.dma_start(out=zt[:, :, :512], in_=z_ext[:, :, :512])
            nc.sync.dma_start(out=at[:], in_=at_ext[:])
            nc.sync.dma_start(out=zt[:, :, 512:], in_=z_ext[:, :, 512:])

            # two planes synthesized on the otherwise-idle Pool engine to
            # shave the DVE stream (Pool is ~5x slower per op but has slack)
            pool_onehots = (9, 15)
            # first planes synthesized per m-half so plane 0's mt=0 matmul
            # only waits on the first z half
            half_onehots = 4
            issued = set()

            def _cmp(q):
                # (z == q) -> 1/0 uint16, * 0x38 -> low byte = fp8(1.0)
                if q in issued or q >= N_PLANES:
                    return
                issued.add(q)
                if q in pool_onehots:
                    nc.gpsimd.tensor_scalar(
                        zq[:, q], zt[:], float(q), _FP8_ONE_BYTE,
                        mybir.AluOpType.is_equal, mybir.AluOpType.mult,
                    )
                elif q < half_onehots:
                    for h in range(2):
                        nc.vector.tensor_scalar(
                            zq[:, q, :, h * 512:(h + 1) * 512],
                            zt[:, :, h * 512:(h + 1) * 512],
                            float(q), _FP8_ONE_BYTE,
                            mybir.AluOpType.is_equal, mybir.AluOpType.mult,
                        )
                else:
                    nc.vector.tensor_scalar(
                        zq[:, q], zt[:], float(q), _FP8_ONE_BYTE,
                        mybir.AluOpType.is_equal, mybir.AluOpType.mult,
                    )

            for q in sorted(pool_onehots):
                _cmp(q)
            lookahead = 4
            for q in range(lookahead):
                _cmp(q)

            ci = 0
            for q in range(N_PLANES):
                for fq in range(q + 1, q + 1 + lookahead):
                    _cmp(fq)
                otile = opool.tile([128, 2, M_LOC], mybir.dt.uint8)
                # stride-2 fp8 view of the uint16 words = the one-hot plane
                zq8 = zq[:, q].bitcast(mybir.dt.float8e4)  # [128, 2, 2048]
                for rb in range(2):
                    acc = ppool.tile([128, 1024], mybir.dt.float32)
                    for mt in range(M_LOC // 512):
                        rhs = zq8[:, :, mt * 1024:(mt + 1) * 1024:2]
                        nc.tensor.matmul(
                            acc[:, mt * 512:(mt + 1) * 512],
                            at[:, :, rb * 128:(rb + 1) * 128],
                            rhs,
                            start=True,
                            stop=True,
                            perf_mode=mybir.MatmulPerfMode.DoubleRow,
                        )
                    # one merged [128,1024] PSUM->SBUF drain per rb that
                    # also quantizes E*QSCALE -> uint8, split 2:1 ACT:DVE
                    # (DVE also synthesizes the one-hots; gpsimd cannot
                    # read PSUM here)
                    if ci % 8 in (2, 5, 7):
                        nc.vector.tensor_scalar(
                            otile[:, rb], acc[:], QSCALE, None,
                            mybir.AluOpType.mult,
                        )
                    else:
                        nc.scalar.activation(
                            otile[:, rb], acc[:],
                            mybir.ActivationFunctionType.Copy, 0.0, QSCALE,
                        )
                    # per-rb output DMAs pipeline the writeback finer;
                    # alternate between the SP/HWDGE queue and the Pool/SWDGE
                    # queue (two parallel DMA issue paths — HWDGE generation
                    # at 625 ns/DMA otherwise congests the writeback tail)
                    if ci % 2 == 0:
                        nc.gpsimd.dma_start(out=e_ext[q, :, rb], in_=otile[:, rb])
                    else:
                        nc.sync.dma_start(out=e_ext[q, :, rb], in_=otile[:, rb])
                    ci += 1
    nc.finalize()   # Bacc.compile(): sync legalization + reg allocation
    _CACHE["fast"] = nc
    return nc


# --------------------------------------------------------------------------
# per-q graph: E[q,r,m] = sum_j Atq[q][r,j] * [Z[j,m]==q], all 21 planes
# (used when the Vaa diagonal is not uniform enough for the shared-Atot
# shortcut)
# --------------------------------------------------------------------------
def _build_perq_graph():
    if "perq" in _CACHE:
        return _CACHE["perq"]
    nc = bacc.Bacc(None, target_bir_lowering=False)
    at_ext = nc.declare_dram_parameter(
        "at", [Q_AA, 128, 2, L], mybir.dt.float16, isOutput=False
    )
    z_ext = nc.declare_dram_parameter(
        "z", [128, 2, M_LOC], mybir.dt.float16, isOutput=False
    )
    e_ext = nc.declare_dram_parameter(
        "e", [Q_AA, 128, 2, M_LOC], mybir.dt.float16, isOutput=True
    )

    with tile.TileContext(nc) as tc:
        with (
            tc.tile_pool(name="const", bufs=1) as cpool,
            tc.tile_pool(name="out", bufs=6) as opool,
            tc.tile_pool(name="psum", bufs=8, space=bass.MemorySpace.PSUM) as ppool,
        ):
            zt = cpool.tile([128, 2, M_LOC], mybir.dt.float16)
            at = cpool.tile([128, Q_AA, 2, L], mybir.dt.float16)
            zq = cpool.tile([128, Q_AA, 2, M_LOC], mybir.dt.float16)

            nc.sync.dma_start(out=zt[:], in_=z_ext[:])
            for q in range(Q_AA):
                nc.sync.dma_start(out=at[:, q], in_=at_ext[q])

            def _cmp(q):
                nc.vector.tensor_scalar(
                    zq[:, q], zt[:], float(q), None, mybir.AluOpType.is_equal
                )

            lookahead = 4
            for q in range(min(lookahead, Q_AA)):
                _cmp(q)

            ci = 0
            for q in range(Q_AA):
                if q + lookahead < Q_AA:
                    _cmp(q + lookahead)
                otile = opool.tile([128, 2, M_LOC], mybir.dt.float16)
                for rb in range(2):
                    for mt in range(M_LOC // 512):
                        acc = ppool.tile([128, 512], mybir.dt.float32)
                        for kb in range(2):
                            nc.tensor.matmul(
                                acc[:],
                                at[:, q, kb, rb * 128:(rb + 1) * 128],
                                zq[:, q, kb, mt * 512:(mt + 1) * 512],
                                start=(kb == 0),
                                stop=(kb == 1),
                            )
                        if ci % 3 == 2:
                            nc.vector.tensor_copy(
                                otile[:, rb, mt * 512:(mt + 1) * 512], acc[:]
                            )
                        else:
                            nc.scalar.copy(
                                otile[:, rb, mt * 512:(mt + 1) * 512], acc[:]
                            )
                        ci += 1
                    nc.sync.dma_start(out=e_ext[q, :, rb], in_=otile[:, rb])
    nc.finalize()
    _CACHE["perq"] = nc
    return nc


# --------------------------------------------------------------------------
# dense fallback graph: E = Jmat @ Zoh  (always valid)
# --------------------------------------------------------------------------
def _build_dense_graph():
    if "dense" in _CACHE:
        return _CACHE["dense"]
    nc = bacc.Bacc(None, target_bir_lowering=False)
    jt_ext = nc.declare_dram_parameter(
        "jt", [NB, 128, NB, 128], mybir.dt.bfloat16, isOutput=False
    )
    zoh_ext = nc.declare_dram_parameter(
        "zoh", [128, NB, M_LOC], mybir.dt.bfloat16, isOutput=False
    )
    out_ext = nc.declare_dram_parameter(
        "out", [F, M_LOC], mybir.dt.float32, isOutput=True
    )

    with tile.TileContext(nc) as tc:
        with (
            tc.tile_pool(name="zpool", bufs=1) as zpool,
            tc.tile_pool(name="jpool", bufs=3) as jpool,
            tc.tile_pool(name="opool", bufs=4) as opool,
            tc.tile_pool(name="psum", bufs=4, space=bass.MemorySpace.PSUM) as ppool,
        ):
            ztile = zpool.tile([128, NB, M_LOC], mybir.dt.bfloat16)
            nc.sync.dma_start(out=ztile[:], in_=zoh_ext[:])

            for i in range(NB):
                jtile = jpool.tile([128, NB, 128], mybir.dt.bfloat16)
                nc.sync.dma_start(out=jtile[:], in_=jt_ext[i])
                for mt in range(M_LOC // 512):
                    acc = ppool.tile([128, 512], mybir.dt.float32)
                    for k in range(NB):
                        nc.tensor.matmul(
                            acc[:],
                            jtile[:, k, :],
                            ztile[:, k, mt * 512:(mt + 1) * 512],
                            start=(k == 0),
                            stop=(k == NB - 1),
                        )
                    otile = opool.tile([128, 512], mybir.dt.float32)
                    nc.vector.tensor_copy(otile[:], acc[:])
                    nc.sync.dma_start(
                        out=out_ext[i * 128:(i + 1) * 128, mt * 512:(mt + 1) * 512],
                        in_=otile[:],
                    )
    nc.finalize()
    _CACHE["dense"] = nc
    return nc


def _softmax(x, axis):
    x = x - x.max(axis=axis, keepdims=True)
    e = np.exp(x)
    return e / e.sum(axis=axis, keepdims=True)


def _epilogue(E, Zi, weights, reg):
    """E: (q, L, M) fp32; pl + reg."""
    mx = E.max(axis=0)
    lge = mx + np.log(np.sum(np.exp(E - mx[None]), axis=0))
    Ec = np.take_along_axis(E, Zi[None], axis=0)[0]
    pl = -float(np.sum(weights * np.sum(Ec - lge, axis=0)))
    return np.float32(pl + reg)


def _reg_exact(A, Vaa):
    """L2 reg from the exact per-q Atq (cheap on host, ~1.4 M elements)."""
    diag = Vaa[:, np.arange(Q_AA), np.arange(Q_AA)]
    Atq = np.einsum("hij,hq->qij", A, diag).astype(np.float32)
    Atq[:, np.arange(L), np.arange(L)] = 0.0
    return Atq, LAMBDA * float(np.sum(Atq.astype(np.float64) ** 2))


def _z_maps(Zi):
    Zf = Zi.astype(np.float16)                             # values 0..20 exact
    zs = []
    for c in range(N_CORES):
        zc = Zf[:, c * M_LOC:(c + 1) * M_LOC]              # (256, 1024)
        zs.append(np.ascontiguousarray(
            zc.reshape(2, 128, M_LOC).transpose(1, 0, 2)
        ))                                                 # (128, 2, 1024)
    return zs


def _fast_path(Atot, reg, Zi, weights):
    """Shared-Atot fp8 20-plane device path + host plane-21 reconstruction."""
    Atot8 = Atot.astype(_FP8_DT)
    at_np = np.ascontiguousarray(
        Atot8.T.reshape(2, 128, L).transpose(1, 0, 2)
    )                                                      # (128, 2, 256) fp8
    in_maps = [{"at": at_np, "z": z} for z in _z_maps(Zi)]

    nc = _build_fast_graph()
    res = run_bass_kernel_spmd(nc, in_maps, list(range(N_CORES)))
    E = np.empty((Q_AA, L, M_TOT), np.float32)
    E[:N_PLANES] = np.concatenate(
        [
            np.asarray(res.results[c]["e"])
            .astype(np.float32)                            # (q, 128, 2, m)
            .transpose(0, 2, 1, 3)                         # (q, rb, 128, m)
            .reshape(N_PLANES, L, M_LOC)
            for c in range(N_CORES)
        ],
        axis=2,
    ) * np.float32(1.0 / QSCALE)                           # dequantize
    # sum_q E_q == rowsum(fp8(Atot)) since the one-hot planes partition (j,m)
    rowsum = Atot8.astype(np.float32).sum(axis=1)          # (L,)
    E[N_PLANES] = rowsum[:, None] - E[:N_PLANES].sum(axis=0)
    return _epilogue(E, Zi, weights, reg)


def _perq_path(Atq, reg, Zi, weights):
    """Per-q 21-plane device path (diag of Vaa not uniform)."""
    AtT = Atq.transpose(0, 2, 1)                           # (q, j, r)
    at_np = np.ascontiguousarray(
        AtT.reshape(Q_AA, 2, 128, L).transpose(0, 2, 1, 3)
    ).astype(np.float16)                                   # (q, 128, 2, L)
    in_maps = [{"at": at_np, "z": z} for z in _z_maps(Zi)]

    nc = _build_perq_graph()
    res = run_bass_kernel_spmd(nc, in_maps, list(range(N_CORES)))
    E = np.concatenate(
        [
            np.asarray(res.results[c]["e"])
            .astype(np.float32)
            .transpose(0, 2, 1, 3)
            .reshape(Q_AA, L, M_LOC)
            for c in range(N_CORES)
        ],
        axis=2,
    )                                                      # (q, L, M)
    return _epilogue(E, Zi, weights, reg)


def _fast_path_host(Atq, reg, Zi, weights):
    """Numpy fallback of the per-q formulation (if device paths fail)."""
    E = np.empty((Q_AA, L, M_TOT), np.float32)
    for q in range(Q_AA):
        E[q] = Atq[q] @ (Zi == q).astype(np.float32)
    return _epilogue(E, Zi, weights, reg)


def _dense_path(A, Vaa, Zi, weights):
    J = (A.reshape(H, L * L).T @ Vaa.reshape(H, Q_AA * Q_AA)).reshape(
        L, L, Q_AA, Q_AA
    )
    J[np.arange(L), np.arange(L)] = 0.0
    reg = LAMBDA * float(np.sum(J.astype(np.float64) ** 2))

    Jmat = np.ascontiguousarray(J.transpose(0, 2, 1, 3).reshape(F, F))
    JT4 = np.ascontiguousarray(Jmat.T).reshape(NB, 128, NB, 128)
    jt_np = np.ascontiguousarray(JT4.transpose(2, 1, 0, 3)).astype(
        ml_dtypes.bfloat16
    )

    colidx = np.arange(L)[:, None] * Q_AA + Zi             # (L, M)
    in_maps = []
    for c in range(N_CORES):
        ci = colidx[:, c * M_LOC:(c + 1) * M_LOC]
        zfull = np.zeros((F, M_LOC), np.float32)
        zfull[ci, np.arange(M_LOC)[None, :]] = 1.0
        zoh_np = np.ascontiguousarray(
            zfull.reshape(NB, 128, M_LOC).transpose(1, 0, 2)
        ).astype(ml_dtypes.bfloat16)
        in_maps.append({"jt": jt_np, "zoh": zoh_np})

    try:
        nc = _build_dense_graph()
        res = run_bass_kernel_spmd(nc, in_maps, list(range(N_CORES)))
        E = np.concatenate(
            [np.asarray(res.results[c]["out"]).astype(np.float32)
             for c in range(N_CORES)], axis=1
        )
    except Exception:
        shards = []
        for c in range(N_CORES):
            ci = colidx[:, c * M_LOC:(c + 1) * M_LOC]
            zfull = np.zeros((F, M_LOC), np.float32)
            zfull[ci, np.arange(M_LOC)[None, :]] = 1.0
            shards.append(Jmat @ zfull)
        E = np.concatenate(shards, axis=1)

    E3 = np.ascontiguousarray(E.reshape(L, Q_AA, M_TOT).transpose(1, 0, 2))
    return _epilogue(E3, Zi, weights, reg)


def kernel(reps_matrix, Q, K, V_metric, Z, weights):
    reps_matrix = np.asarray(reps_matrix, np.float32)
    Q = np.asarray(Q, np.float32)
    K = np.asarray(K, np.float32)
    V_metric = np.asarray(V_metric, np.float32)
    Zi = np.asarray(Z).astype(np.int64)
    weights = np.asarray(weights, np.float32)

    # --- host prologue: attention map + RBF kernel ---
    scores = np.einsum("hid,hjd->hij", Q, K) / np.sqrt(np.float32(DK))
    probs = _softmax(scores, axis=-1)
    A = 0.5 * (probs + probs.transpose(0, 2, 1))           # (H, L, L)

    V1 = np.einsum("qd,hdv->hqv", reps_matrix, V_metric)   # (H, q, dv)
    gamma = 1.0 / V1.shape[1]
    sq = np.sum(V1 * V1, axis=-1)
    D2 = sq[:, :, None] + sq[:, None, :] - 2.0 * np.einsum("hqv,hav->hqa", V1, V1)
    Vaa = np.exp(-gamma * np.maximum(D2, 0.0))             # (H, q, q)

    # dropped-residual bound for the fast paths: |E_res| <= max_offdiag(Vaa)
    # * max row-sum of sum_h A[h]
    offmax = float((Vaa * (1.0 - np.eye(Q_AA, dtype=np.float32))[None]).max())
    rowsum = float(np.abs(A).sum(axis=0).sum(axis=1).max())
    diagdev = float(
        np.abs(Vaa[:, np.arange(Q_AA), np.arange(Q_AA)] - 1.0).max()
    )
    if offmax * rowsum < 1e-7:
        Atq, reg = _reg_exact(A, Vaa)
        try:
            Atot = A.sum(axis=0).astype(np.float32)
            Atot[np.arange(L), np.arange(L)] = 0.0
            rs_max = float(Atot.astype(_FP8_DT).astype(np.float32).sum(1).max())
            if diagdev * rowsum < 1e-2 and rs_max * QSCALE < 250.0:
                # Vaa ~ I enough to share one Atot, and E fits uint8 range
                return _fast_path(Atot, reg, Zi, weights)
            return _perq_path(Atq, reg, Zi, weights)
        except Exception:
            return _fast_path_host(Atq, reg, Zi, weights)
    return _dense_path(A, Vaa, Zi, weights)
